# revision 30
# baseline (speedup 1.0000x reference)
"""BiDirectionalMinGRU Trainium2 kernel (v2).

Strategy
--------
Data-parallel over batch: 16 samples / 8 cores = 2 samples per core,
processed sequentially, weights replicated.  The minGRU log-space scan is
computed as the equivalent linear recurrence h_t = a_t*h_{t-1} + b_t with
a = sigmoid(-k) (fp32 - 20% of gates have a > 0.9999, bf16 would destroy
the decay rates) and b = sigmoid(k)*g(v) (bf16-safe).

Work placement per (chunk, direction, tile):
  PE  : k/v projection matmuls (f32r, contraction 10), LN-stats matmuls
        (indicator-column stationaries accumulating [16,T] in PSUM), MLP
        head matmuls (bf16).
  ACT : the two sigmoids, the gelu (Gelu_apprx_tanh table).
  DVE : custom fused ops GATE_G (select of g's two branches straight from
        the v PSUM tile) and GATE_B ((1-a)*g), forward scans, h^2 squares.
  Pool: backward scans (reversed-AP reads), r broadcast, y copies.

The time-encoding MLP depends only on input t, so te, sum(te), sum(te^2)
are computed on the host and shipped as inputs; the final +b2 bias is also
applied on the host.  LayerNorm is folded into the head matmul via the
(-mu)*colsum extra contraction row.
"""

import sys

sys.path.insert(0, "/opt/trn_rl_repo")

import numpy as np
import ml_dtypes

import concourse.bass as bass
import concourse.tile as tile
from concourse import mybir
from concourse.mybir import AluOpType as alu

AF = mybir.ActivationFunctionType
F32 = mybir.dt.float32
F32R = mybir.dt.float32r
BF16 = mybir.dt.bfloat16
BF = ml_dtypes.bfloat16

# problem dims (hardcoded; harness always calls with these shapes)
B, L, H = 16, 8192, 256
TE = 8
RIN = 10
OUT = 2 * H + TE  # 520
HH = 128
N_CORES = 8
SPC = B // N_CORES  # samples per core = 2
T = 512            # time tile
NT = L // T        # 16 tiles

E5 = float(np.exp(np.float32(5.0)))
EPS = 1e-5
DEBUG_DUMP = False

# ---------------------------------------------------------------------------
# custom DVE ops: registered into the concourse dve_ops registry at import.
# ---------------------------------------------------------------------------


def _register_gate_ops():
    import concourse.dve_ops as dve_ops
    from concourse.dve_spec import Spec, Src0, Src1, C0, C1, C2, select, lower
    from concourse.dve_spec import _has_src1
    from concourse.dve_uop import DveOpSpec

    if "GATE_G_ANT" in dve_ops._SUB_OPCODE_FOR_NAME:
        by_name = {op.name: op for op in dve_ops.OPS}
        return by_name["GATE_G_ANT"], by_name["GATE_B_ANT"]

    _y = Src0 + C0
    g_spec = Spec(
        body=select(_y >= C1, _y, Src1 * C2),
        reference=lambda in0, in1, s0, s1, imm2: np.where(
            (in0.astype(np.float32) + s0) >= s1,
            in0.astype(np.float32) + s0,
            in1.astype(np.float32) * imm2,
        ).astype(np.float32),
    )
    b_spec = Spec(
        body=(C0 - Src0) * Src1,
        reference=lambda in0, in1, s0, s1, imm2: (
            (s0 - in0.astype(np.float32)) * in1.astype(np.float32)
        ).astype(np.float32),
    )

    made = []
    for name, spec in (("GATE_G_ANT", g_spec), ("GATE_B_ANT", b_spec)):
        row = dve_ops._CUSTOM_DVE_ROW_BASE + len(dve_ops.OPS)
        shas = {}
        for ver in ("v3", "v4"):
            s = DveOpSpec(
                name=name, opcode=row, uops=lower(spec, ver=ver),
                rd1_en=_has_src1(spec),
            )
            shas[ver] = s.sha(ver)
        op = dve_ops.DveOp(name=name, spec=spec, subdim=False, uops_sha=shas)
        dve_ops.OPS.append(op)
        dve_ops._SUB_OPCODE_FOR_NAME[name] = row
        dve_ops.CUSTOM_DVE_SPECS[name] = spec
        made.append(op)
    return made[0], made[1]


GATE_G_OP, GATE_B_OP = _register_gate_ops()

# fp32 const blob layout: name -> (partitions, col offset, width)
BLOBF_LAYOUT = {
    "nckf": (128, 0, 2), "chf": (128, 2, 2), "chpf": (128, 4, 2),
    "nckb": (128, 6, 2), "chb": (128, 8, 2), "chpb": (128, 10, 2),
    "b1p": (HH, 12, 1), "eps16": (16, 13, 1),
    "ckf": (128, 14, 2), "ckb": (128, 16, 2),
    "nchf": (128, 18, 2), "nchb": (128, 20, 2),
}
BLOBF_W = 22
# bf16 const blob layout
BLOBB_LAYOUT = {
    "w1c0": (128, 0, HH), "w1c1": (128, 128, HH), "w1c2": (128, 256, HH),
    "w1c3": (128, 384, HH), "w1cte": (TE, 512, HH),
    "augw": (1, 640, HH),
    "indones": (128, 768, 16 * NT),   # stationary j: cols j*16..j*16+15, col j ones
    "w2ind": (128, 1024, 16 * NT),    # stationary j: col j = gh_w2 vector
    "onesr": (1, 1280, 128),
}
BLOBB_W = 1408


def build_core_program():
    """Build the per-core Bass program (2 samples, sequential)."""
    nc = bass.Bass()

    rnn_d = nc.dram_tensor("rnn", [SPC, 3 * RIN, L], BF16, kind="ExternalInput")
    teb_d = nc.dram_tensor("teb", [SPC, TE, L], BF16, kind="ExternalInput")
    st_d = nc.dram_tensor("st", [SPC, 16, 2 * T], F32, kind="ExternalInput")
    blobw_d = nc.dram_tensor("blobw", [3 * RIN, 1024], BF16, kind="ExternalInput")
    blobf_d = nc.dram_tensor("blobf", [128, BLOBF_W], F32, kind="ExternalInput")
    blobb_d = nc.dram_tensor("blobb", [128, BLOBB_W], BF16, kind="ExternalInput")
    y_d = nc.dram_tensor("y", [SPC, NT, T], F32, kind="ExternalOutput")
    dbg = {}
    if DEBUG_DUMP:
        for s in range(SPC):
            for nm in ("hf0", "hf1", "hb0", "hb1"):
                dbg[f"{nm}_s{s}"] = nc.dram_tensor(f"dbg_{nm}_s{s}", [128, L],
                                                   BF16, kind="ExternalOutput")
            dbg[f"mun_s{s}"] = nc.dram_tensor(f"dbg_mun_s{s}", [NT, T], F32,
                                              kind="ExternalOutput")
            dbg[f"r16_s{s}"] = nc.dram_tensor(f"dbg_r16_s{s}", [NT, T], F32,
                                              kind="ExternalOutput")

    with tile.TileContext(nc) as tc:
        _emit(tc, dict(rnn=rnn_d, teb=teb_d, st=st_d, blobf=blobf_d,
                       blobw=blobw_d, blobb=blobb_d, y=y_d, dbg=dbg))
    return _split_sync_waits(nc)


def _emit(tc, d):
    nc = tc.nc
    with tc.tile_pool(name="const", bufs=1) as const:
        blobf = const.tile([128, BLOBF_W], F32, tag="blobf", name="blobf")
        nc.sync.dma_start(blobf[:], d["blobf"][:])
        blobb = const.tile([128, BLOBB_W], BF16, tag="blobb", name="blobb")
        nc.sync.dma_start(blobb[:], d["blobb"][:])
        blobw = const.tile([3 * RIN, 1024], BF16, tag="blobw", name="blobw")
        nc.sync.dma_start(blobw[:], d["blobw"][:])

        def cs(name):
            p, off, w = BLOBF_LAYOUT[name]
            return blobf[0:p, off:off + w]

        def csb(name):
            p, off, w = BLOBB_LAYOUT[name]
            return blobb[0:p, off:off + w]

        c = dict(
            wkf=blobw[:, 0:256], whf=blobw[:, 256:512],
            wkb=blobw[:, 512:768], whb=blobw[:, 768:1024],
            nckf=cs("nckf"), chf=cs("chf"), chpf=cs("chpf"),
            nckb=cs("nckb"), chb=cs("chb"), chpb=cs("chpb"),
            ckf=cs("ckf"), ckb=cs("ckb"), nchf=cs("nchf"), nchb=cs("nchb"),
            b1p=cs("b1p"), eps16=cs("eps16"),
            w1chunks=[csb("w1c0"), csb("w1c1"), csb("w1c2"), csb("w1c3"),
                      csb("w1cte")],
            augw=csb("augw"), indones=csb("indones"), w2ind=csb("w2ind"),
            onesr=csb("onesr"),
        )

        with tc.tile_pool(name="work", bufs=2) as work, \
             tc.tile_pool(name="sbuf", bufs=1) as sbuf, \
             tc.tile_pool(name="wps", bufs=2, space="PSUM") as wps, \
             tc.tile_pool(name="acc", bufs=1, space="PSUM") as accp:
            # warmup: one PE touch of each const blob so later matmuls carry
            # at most one new semaphore wait (the LW slot fits only one).
            wu = accp.tile([2, 2], F32, tag="wu", name="warmup")
            nc.tensor.matmul(wu[:], blobb[0:1, 0:2], blobb[0:1, 0:2],
                             start=True, stop=False, skip_group_check=True)
            nc.tensor.matmul(wu[:], blobf[0:1, 0:1].bitcast(BF16),
                             blobf[0:1, 0:1].bitcast(BF16),
                             start=False, stop=False, skip_group_check=True)
            nc.tensor.matmul(wu[:], blobw[0:1, 0:2], blobw[0:1, 0:2],
                             start=False, stop=True, skip_group_check=True)
            # ACT/DVE queue warmups: wait each const-blob DMA semaphore once,
            # alone (compute instructions cannot mix a DMA wait with others).
            wsc = work.tile([1, 1], F32, tag="wsc", name="wsc", bufs=1)
            nc.scalar.copy(wsc[:], blobf[0:1, 0:1])
            wsv = work.tile([1, 1], F32, tag="wsv", name="wsv", bufs=1)
            nc.vector.tensor_scalar(wsv[:], blobf[0:1, 0:1], 0.0, None, alu.add)
            # sample tiles are shared between the two (sequential) samples:
            # the second sample's DMAs/scans overwrite them, so its matmuls
            # wait on a single producer semaphore instead of released-pool
            # overlap dependencies.
            tiles = dict(
                hf=[sbuf.tile([128, L], BF16, tag=f"hf{k}", name=f"hf{k}")
                    for k in (0, 1)],
                hb=[sbuf.tile([128, L], BF16, tag=f"hb{k}", name=f"hb{k}")
                    for k in (0, 1)],
                rnn=sbuf.tile([3 * RIN, L], BF16, tag="rnn", name="rnn"),
                teb=sbuf.tile([TE, L], BF16, tag="teb", name="teb"),
                st16=sbuf.tile([16, 2 * T], F32, tag="st16", name="st16"),
                mu_row=sbuf.tile([1, L], BF16, tag="mu_row", name="mu_row"),
                r_row=sbuf.tile([1, L], BF16, tag="r_row", name="r_row"),
                mun=sbuf.tile([NT, T], F32, tag="mun", name="mun"),
            )
            for s in range(SPC):
                _emit_sample(tc, d, s, c, work, wps, accp, tiles)


def _gate(nc, work, pp, c, direction, ch, rnn_mov, out_ap, init, scan_engine,
          rev):
    """One (direction, chunk) gate pipeline for one tile."""
    csl = slice(ch * 128, (ch + 1) * 128)
    wk = c["wkf" if direction == "f" else "wkb"]
    wh = c["whf" if direction == "f" else "whb"]
    nck = c["nckf" if direction == "f" else "nckb"]
    ck = c["ckf" if direction == "f" else "ckb"]
    chv = c["chf" if direction == "f" else "chb"]
    chp = c["chpf" if direction == "f" else "chpb"]
    nch = c["nchf" if direction == "f" else "nchb"]

    k_ps = pp.tile([128, T], F32, tag="k_ps", name="k_ps")
    nc.tensor.matmul(k_ps[:], wk[:, csl], rnn_mov,
                     start=True, stop=True)
    v_ps = pp.tile([128, T], F32, tag="v_ps", name="v_ps")
    nc.tensor.matmul(v_ps[:], wh[:, csl], rnn_mov,
                     start=True, stop=True)
    # sgk = sigmoid(k + ck): relatively accurate where small, which is what
    # the decay rate 1-a needs; a = 1 - sgk in fp32 (the direct sigmoid(-k)
    # table value near 1 has only absolute accuracy - fatal for 1-a).
    sgk = work.tile([128, T], F32, tag="sgk", name="sgk")
    nc.scalar.activation(sgk[:], k_ps[:], AF.Sigmoid, bias=ck[:, ch:ch + 1])
    a = work.tile([128, T], F32, tag="a", name="a")
    nc.vector.tensor_scalar(a[:], sgk[:], -1.0, 1.0, alu.mult, alu.add)
    # sgm = sigmoid(v + ch) in bf16
    sgm = work.tile([128, T], BF16, tag="sgm", name="sgm")
    nc.scalar.activation(sgm[:], v_ps[:], AF.Sigmoid, bias=chv[:, ch:ch + 1])
    # vp = v + ch + 0.5 (value only; the branch predicate is fp32-exact below)
    vp = work.tile([128, T], BF16, tag="vp", name="vp")
    nc.scalar.activation(vp[:], v_ps[:], AF.Identity, bias=chp[:, ch:ch + 1])
    # mge = (v + ch >= 0), compared in fp32 straight from PSUM
    mge = work.tile([128, T], mybir.dt.uint8, tag="mge", name="mge")
    nc.vector.tensor_scalar(mge[:], v_ps[:], nch[:, ch:ch + 1], None, alu.is_ge)
    # g = mge ? vp : e5*sgm
    g = work.tile([128, T], BF16, tag="g", name="g")
    nc.vector.tensor_scalar(g[:], sgm[:], E5, None, alu.mult)
    nc.vector.copy_predicated(g[:], mge[:], vp[:])
    # b = sigmoid(k) * g
    b = work.tile([128, T], BF16, tag="b", name="b")
    nc.vector.tensor_tensor(b[:], sgk[:], g[:], alu.mult)
    if rev:
        scan_engine.tensor_tensor_scan(out_ap, a[:, ::-1], b[:, ::-1], init,
                                       alu.mult, alu.add)
    else:
        scan_engine.tensor_tensor_scan(out_ap, a[:], b[:], init,
                                       alu.mult, alu.add)


def _emit_sample(tc, d, s, c, work, wps, accp, tiles):
    nc = tc.nc
    if True:
        hf = tiles["hf"]
        hb = tiles["hb"]
        rnn = tiles["rnn"]
        nc.sync.dma_start(rnn[:], d["rnn"][s])
        teb = tiles["teb"]
        nc.sync.dma_start(teb[:], d["teb"][s])
        st16 = tiles["st16"]
        nc.sync.dma_start(st16[:], d["st"][s])
        wst = work.tile([1, 1], F32, tag="wst", name=f"wst_s{s}", bufs=1)
        nc.vector.tensor_scalar(wst[:], st16[0:1, 0:1], 0.0, None, alu.add)
        mu_row = tiles["mu_row"]
        r_row = tiles["r_row"]

        # ---------------- pass 1: forward gates + scans (DVE) --------------
        for j in range(NT):
            sl = slice(j * T, (j + 1) * T)
            for ch in (0, 1):
                init = 0.5 if j == 0 else hf[ch][:, j * T - 1:j * T]
                _gate(nc, work, wps, c, "f", ch, rnn[:, sl],
                      hf[ch][:, sl], init, nc.vector, rev=False)

        # ------- pass 2: backward gates + scans (Pool) + stats matmuls ------
        s1_acc = accp.tile([NT, T], F32, tag="s1_acc", name=f"s1_acc_s{s}")
        s2_acc = accp.tile([NT, T], F32, tag="s2_acc", name=f"s2_acc_s{s}")
        for jj in range(NT):
            tj = NT - 1 - jj
            lo, hi = tj * T, (tj + 1) * T
            for ch in (0, 1):
                init = 0.5 if jj == 0 else hb[ch][:, hi:hi + 1]
                out_ap = hb[ch][:, lo:hi][:, ::-1]
                _gate(nc, work, wps, c, "b", ch, rnn[:, lo:hi],
                      out_ap, init, nc.vector, rev=True)

            # LN stats for range [lo:hi) -> accumulated rows tj
            ind = c["indones"][:, tj * 16:(tj + 1) * 16]
            Xs = [hf[0][:, lo:hi], hf[1][:, lo:hi],
                  hb[0][:, lo:hi], hb[1][:, lo:hi]]
            for i4, xt in enumerate(Xs):
                nc.tensor.matmul(s1_acc[:], ind, xt,
                                 start=(jj == 0 and i4 == 0), stop=False,
                                 skip_group_check=True)
            for i4, xt in enumerate(Xs):
                sq = work.tile([128, T], BF16, tag=f"sq{i4}", name="sq")
                nc.vector.tensor_tensor(sq[:], xt, xt, alu.mult)
                nc.tensor.matmul(s2_acc[:], ind, sq[:],
                                 start=(jj == 0 and i4 == 0), stop=False,
                                 skip_group_check=True)
        # close the accumulation groups with a zero contribution
        zb = work.tile([1, T], BF16, tag="zb", name="zb", bufs=1)
        nc.gpsimd.memset(zb[:], 0.0)
        nc.tensor.matmul(s1_acc[:], c["indones"][0:1, 0:16], zb[:],
                         start=False, stop=True, skip_group_check=True)
        nc.tensor.matmul(s2_acc[:], c["indones"][0:1, 0:16], zb[:],
                         start=False, stop=True, skip_group_check=True)

        # ---------------- batched LN stats finalize ----------------
        # mun = -(s1h + s1te)/OUT ; st16 rows 0:16 hold -s1te/OUT
        mun = tiles["mun"]
        nc.vector.scalar_tensor_tensor(mun[:], s1_acc[:], -1.0 / OUT,
                                       st16[:, 0:T], alu.mult, alu.add)
        # e2 = (s2h + s2te)/OUT ; st16 rows 16:32 hold +s2te/OUT
        e2 = work.tile([NT, T], F32, tag="e2", name="e2", bufs=1)
        nc.vector.scalar_tensor_tensor(e2[:], s2_acc[:], 1.0 / OUT,
                                       st16[:, T:2 * T], alu.mult, alu.add)
        mu2 = work.tile([NT, T], F32, tag="mu2", name="mu2", bufs=1)
        nc.vector.tensor_tensor(mu2[:], mun[:], mun[:], alu.mult)
        varb = work.tile([NT, T], F32, tag="varb", name="varb", bufs=1)
        nc.vector.tensor_tensor(varb[:], e2[:], mu2[:], alu.subtract)
        lnv = work.tile([NT, T], F32, tag="lnv", name="lnv", bufs=1)
        nc.scalar.activation(lnv[:], varb[:], AF.Ln, bias=c["eps16"][:, 0:1])
        r16 = work.tile([NT, T], BF16, tag="r16", name="r16", bufs=1)
        nc.scalar.activation(r16[:], lnv[:], AF.Exp, scale=-0.5)
        if DEBUG_DUMP:
            dbg = d["dbg"]
            for nm, buf in (("hf0", hf[0]), ("hf1", hf[1]),
                            ("hb0", hb[0]), ("hb1", hb[1])):
                nc.sync.dma_start(dbg[f"{nm}_s{s}"][:], buf[:])
            nc.sync.dma_start(dbg[f"mun_s{s}"][:], mun[:])
            r16d = work.tile([NT, T], F32, tag="r16d", name="r16d", bufs=1)
            nc.vector.tensor_copy(r16d[:], r16[:])
            nc.sync.dma_start(dbg[f"r16_s{s}"][:], r16d[:])
        mun_b = work.tile([NT, T], BF16, tag="mun_b", name="mun_b", bufs=1)
        nc.vector.tensor_copy(mun_b[:], mun[:])
        nc.sync.dma_start(mu_row[0:1, :].rearrange("p (j t) -> p j t", t=T),
                          mun_b[:])
        nc.sync.dma_start(r_row[0:1, :].rearrange("p (j t) -> p j t", t=T),
                          r16[:])
        wpr = work.tile([1, 1], BF16, tag="wpr", name=f"wpr_s{s}", bufs=1)
        nc.gpsimd.tensor_scalar(wpr[:], r_row[0:1, 0:1], 0.0, None, alu.add)

        # ---------------- pass 3: MLP head ----------------
        y_acc = accp.tile([NT, T], F32, tag="y_acc", name=f"y_acc_s{s}")
        for j in range(NT):
            sl = slice(j * T, (j + 1) * T)
            Xs = [hf[0][:, sl], hf[1][:, sl], hb[0][:, sl], hb[1][:, sl],
                  teb[:, sl]]
            m_ps = wps.tile([128, T], F32, tag="k_ps", name="m_ps")
            for i4, (wc, xt) in enumerate(zip(c["w1chunks"], Xs)):
                nc.tensor.matmul(m_ps[:], wc, xt, start=(i4 == 0),
                                 stop=False)
            nc.tensor.matmul(m_ps[:], c["augw"], mu_row[0:1, sl],
                             start=False, stop=True)
            r_ps = wps.tile([128, T], F32, tag="v_ps", name="r_ps")
            nc.tensor.matmul(r_ps[:], c["onesr"], r_row[0:1, sl],
                             start=True, stop=True)
            rb = work.tile([128, T], BF16, tag="rb", name="rb")
            nc.scalar.copy(rb[:], r_ps[:])
            zr = work.tile([128, T], BF16, tag="zr", name="zr")
            nc.vector.tensor_tensor(zr[:], m_ps[:], rb[:], alu.mult)
            gel = work.tile([128, T], BF16, tag="gel", name="gel")
            nc.scalar.activation(gel[:], zr[:], AF.Gelu_apprx_tanh,
                                 bias=c["b1p"][:, 0:1])
            nc.tensor.matmul(y_acc[:], c["w2ind"][:, j * 16:(j + 1) * 16],
                             gel[:], start=(j == 0), stop=(j == NT - 1),
                             skip_group_check=True)
        y_sb = work.tile([NT, T], F32, tag="y_sb", name="y_sb", bufs=1)
        nc.scalar.copy(y_sb[:], y_acc[:])
        nc.sync.dma_start(d["y"][s], y_sb[:])




def _split_sync_waits(nc):
    """Rewrite the module so no instruction carries more than one sync wait
    or update: this walrus build can embed only a single semaphore op per
    ISA struct.  Extra waits move to single-wait NoOps inserted just before
    the instruction on the same engine queue; extra updates move to a NoOp
    just after it."""
    n = [0]

    def mknop(engine, waits, updates):
        n[0] += 1
        nop = mybir.InstNoOp(name=f"I-SW{n[0]}", ins=[], outs=[])
        nop.engine = engine
        nop.sync_info = mybir.SyncInfo(on_wait=waits, on_update=updates)
        return nop

    for f in nc.m.functions:
        for blk in f.blocks:
            out = []
            for inst in blk.instructions:
                si = inst.sync_info
                if si is None:
                    out.append(inst)
                    continue
                waits = list(si.on_wait or [])
                updates = list(si.on_update or [])
                while len(waits) > 1:
                    out.append(mknop(inst.engine, [waits.pop(0)], []))
                post = []
                # DMA completion updates fire from hardware; never move them.
                while inst.opcode != "DMACopy" and len(updates) > 1:
                    post.append(mknop(inst.engine, [], [updates.pop()]))
                inst.sync_info = mybir.SyncInfo(on_wait=waits, on_update=updates)
                out.append(inst)
                out.extend(post)
            blk.instructions = out
    return nc


_CACHED_NC = None


def _get_nc():
    global _CACHED_NC
    if _CACHED_NC is None:
        _CACHED_NC = build_core_program()
    return _CACHED_NC


def host_prep(inputs):
    """Fold weights and the te-MLP on the host."""
    f32 = np.float32
    g = {k: np.asarray(v, dtype=f32) for k, v in inputs.items()}

    # te encoding from t (host): h1 = relu(tsh*w1+b1); te = h1@w2+b2
    tsh = ((g["t"] - g["t"][:, :1]) / f32(inputs["time_scale"])).astype(f32)
    h1 = np.maximum(tsh[..., None] * g["te_w1"][0] + g["te_b1"], 0.0)
    te = (h1 @ g["te_w2"] + g["te_b2"]).astype(f32)          # (B, L, 8)
    s1te = te.sum(-1).astype(f32)                             # (B, L)
    s2te = (te * te).sum(-1).astype(f32)

    xm = (g["x"] * g["mask"][..., None]).astype(f32)          # (B, L, 2)

    # device rnn row order: [te(0:8); x(8:10)]; reference rnn = [x(2); te(8)]
    perm = np.array([2, 3, 4, 5, 6, 7, 8, 9, 0, 1])

    def fold(proj_w, proj_b, wz, bz, wh, bh):
        Wk = (proj_w @ wz).astype(f32)[perm]
        ck = (proj_b @ wz + bz).astype(f32)
        Wh = (proj_w @ wh).astype(f32)[perm]
        chv = (proj_b @ wh + bh).astype(f32)
        return Wk, ck, Wh, chv

    Wkf, ckf, Whf, chf = fold(g["fproj_w"], g["fproj_b"], g["f_wz"],
                              g["f_bz"], g["f_wh"], g["f_bh"])
    Wkb, ckb, Whb, chb = fold(g["bproj_w"], g["bproj_b"], g["b_wz"],
                              g["b_bz"], g["b_wh"], g["b_bh"])

    def cols(v):  # (256,) -> (128, 2), column c = chunk c
        return np.ascontiguousarray(v.reshape(2, 128).T)

    W1g = (g["ln_g"][:, None] * g["gh_w1"]).astype(f32)
    W1g_bf = W1g.astype(BF)
    colsum = W1g_bf.astype(f32).sum(0)
    b1p = (g["gh_b1"] + g["ln_b"] @ g["gh_w1"]).astype(f32)

    blobf = np.zeros((128, BLOBF_W), dtype=f32)
    Wcat = np.concatenate([Wkf, Whf, Wkb, Whb], axis=1).astype(f32)  # (10,1024)
    W_hi = Wcat.astype(BF).astype(f32)
    W_lo = (Wcat - W_hi).astype(BF)
    blobw = np.concatenate([W_hi.astype(BF), W_hi.astype(BF), W_lo], axis=0)  # (30,1024)

    def put(name, val):
        p, off, w = BLOBF_LAYOUT[name]
        assert val.shape == (p, w), (name, val.shape)
        blobf[0:p, off:off + w] = val

    put("nckf", cols(-ckf)); put("chf", cols(chf)); put("chpf", cols(chf + 0.5))
    put("nckb", cols(-ckb)); put("chb", cols(chb)); put("chpb", cols(chb + 0.5))
    put("ckf", cols(ckf)); put("ckb", cols(ckb))
    put("nchf", cols(-chf)); put("nchb", cols(-chb))
    put("b1p", b1p.reshape(HH, 1))
    put("eps16", np.full((16, 1), EPS, dtype=f32))

    blobb = np.zeros((128, BLOBB_W), dtype=BF)

    def putb(name, val):
        p, off, w = BLOBB_LAYOUT[name]
        assert val.shape == (p, w), (name, val.shape)
        blobb[0:p, off:off + w] = val.astype(BF)

    for i in range(4):
        putb(f"w1c{i}", W1g_bf[i * 128:(i + 1) * 128, :])
    putb("w1cte", W1g_bf[512:520, :])
    putb("augw", colsum.reshape(1, HH))
    ind = np.zeros((128, 16 * NT), dtype=f32)
    w2i = np.zeros((128, 16 * NT), dtype=f32)
    for j in range(NT):
        ind[:, j * 16 + j] = 1.0
        w2i[:, j * 16 + j] = g["gh_w2"].reshape(-1)
    putb("indones", ind)
    putb("w2ind", w2i)
    putb("onesr", np.ones((1, 128), dtype=f32))

    return dict(blobf=blobf, blobw=blobw, blobb=blobb), te, s1te, s2te, xm


def make_in_maps(inputs):
    wmap, te, s1te, s2te, xm = host_prep(inputs)
    f32 = np.float32
    rnn_f32 = np.concatenate([te, xm], axis=-1).astype(f32)   # (B, L, 10)
    rnn_hi = rnn_f32.astype(BF).astype(f32)
    rnn_lo = (rnn_f32 - rnn_hi).astype(BF)
    # rows: [hi(10); lo(10); hi(10)] pairing stationary [W_hi; W_hi; W_lo]
    rnn_all = np.concatenate([rnn_hi.astype(BF), rnn_lo, rnn_hi.astype(BF)],
                             axis=-1)                          # (B, L, 30)
    in_maps = []
    for i in range(N_CORES):
        m = dict(wmap)
        rnn_c = np.empty((SPC, 3 * RIN, L), BF)
        teb_c = np.empty((SPC, TE, L), BF)
        st_c = np.empty((SPC, 16, 2 * T), f32)
        for s in range(SPC):
            bidx = i * SPC + s
            rnn_c[s] = rnn_all[bidx].T
            teb_c[s] = te[bidx].T.astype(BF)
            st_c[s, :, 0:T] = (-s1te[bidx] / OUT).reshape(NT, T)
            st_c[s, :, T:2 * T] = (s2te[bidx] / OUT).reshape(NT, T)
        m["rnn"] = np.ascontiguousarray(rnn_c)
        m["teb"] = np.ascontiguousarray(teb_c)
        m["st"] = np.ascontiguousarray(st_c)
        in_maps.append(m)
    return in_maps


def _kernel_host(inputs):
    """Validated host fallback: same linear-recurrence formulation (numpy)."""
    f32 = np.float32
    g = {k: np.asarray(v, dtype=f32) for k, v in inputs.items()}

    def sig(z):
        out = np.exp(-np.abs(z))
        return np.where(z >= 0, 1.0 / (1.0 + out), out / (1.0 + out))

    xm = g["x"] * g["mask"][..., None]
    tshv = (g["t"] - g["t"][:, :1]) / g["time_scale"]
    h1 = np.maximum(tshv[..., None] * g["te_w1"][0] + g["te_b1"], 0.0)
    t_enc = (h1 @ g["te_w2"] + g["te_b2"]).astype(f32)
    rnn = np.concatenate([xm, t_enc], axis=-1)

    def scan(pw, pb, wz, bz, wh, bh, reverse):
        k = (rnn @ (pw @ wz) + (pb @ wz + bz)).astype(f32)
        v = (rnn @ (pw @ wh) + (pb @ wh + bh)).astype(f32)
        a = sig(-k)
        bv = sig(k) * np.where(v >= 0, v + 0.5, f32(np.exp(5.0)) * sig(v))
        if reverse:
            a = a[:, ::-1]; bv = bv[:, ::-1]
        h = np.empty_like(a)
        st = np.full((B, H), 0.5, dtype=f32)
        for i in range(L):
            st = a[:, i] * st + bv[:, i]
            h[:, i] = st
        return h[:, ::-1] if reverse else h

    hf = scan(g["fproj_w"], g["fproj_b"], g["f_wz"], g["f_bz"], g["f_wh"],
              g["f_bh"], False)
    hb = scan(g["bproj_w"], g["bproj_b"], g["b_wz"], g["b_bz"], g["b_wh"],
              g["b_bh"], True)
    X = np.concatenate([hf, hb, t_enc], axis=-1)
    mu = X.mean(-1, keepdims=True)
    var = ((X - mu) ** 2).mean(-1, keepdims=True)
    Xn = (X - mu) / np.sqrt(var + 1e-5) * g["ln_g"] + g["ln_b"]
    z = Xn @ g["gh_w1"] + g["gh_b1"]
    gel = 0.5 * z * (1.0 + np.tanh(f32(np.sqrt(2 / np.pi))
                                   * (z + f32(0.044715) * z ** 3)))
    return (gel @ g["gh_w2"] + g["gh_b2"]).astype(f32)


def kernel(**inputs) -> np.ndarray:
    from concourse.bass_utils import run_bass_kernel_spmd

    nc = _get_nc()
    in_maps = make_in_maps(inputs)
    res = run_bass_kernel_spmd(nc, in_maps, list(range(N_CORES)))
    y = np.concatenate([res.results[i]["y"].reshape(SPC, L)
                        for i in range(N_CORES)], axis=0)
    b2 = np.float32(np.asarray(inputs["gh_b2"]).reshape(-1)[0])
    return (y + b2).reshape(B, L, 1).astype(np.float32)


if __name__ == "__main__":
    nc = build_core_program()
    print("built program")


# revision 64
# speedup vs baseline: 1.3573x; 1.3573x over previous
"""BiDirectionalMinGRU Trainium2 kernel.

Strategy
--------
Data-parallel over batch: 16 samples / 8 cores = 2 samples per core,
processed sequentially, weights replicated.  The minGRU log-space scan is
computed as the equivalent linear recurrence h_t = a_t*h_{t-1} + b_t with
a = 1 - sigmoid(k) kept in fp32 (20% of gates sit above 0.9999; bf16 or
the sigmoid(-k) table's absolute error would destroy the decay rates) and
b = sigmoid(k)*g(v) in bf16.

Key design points:
 - Gate projections run as ONE bf16 matmul per (k/v, chunk) with 32
   contraction rows [rnn_hi; rnn_lo; 1; 1; rnn_hi] against
   [W_hi; W_hi; c_hi; c_lo; W_lo]: fp32-grade accuracy at bf16 PE speed
   (1 cyc/col), with the gate biases folded in, so k/v arrive in PSUM
   fully biased and the wide [128, 2T] ACT/DVE/Pool ops need no
   per-chunk bias (f32r was tried first: its precision broke the
   saturated decay channels).
 - Engine placement per (direction, tile): PE matmuls; ACT sigmoid(k),
   v+0.5, sigmoid(v); Pool a = 1-sgk and the exact-fp32 branch predicate
   (v+0.5 >= 0.5); DVE e5-scale, copy_predicated select of g's branches,
   b-multiply, and both directions' scans (reversed APs for backward).
 - LN stats accumulate into PSUM via indicator-column stationaries, split
   into two 8-tile halves so the upper half of the MLP head overlaps the
   backward sweep; mean is folded into the head matmul as a (-mu)*colsum
   contraction row; the te encoding, its stats, and the +b2 bias are
   computed on the host (te depends only on input t).
 - This walrus build encodes at most ONE semaphore wait per instruction:
   _split_sync_waits() rewrites the module, moving extra waits onto
   single-wait NoOps, and "warmup" reads make each queue wait each DMA
   semaphore alone.  Custom DVE ops / partition_broadcast / Pool scans
   all fail codegen here and are not used.
"""

import sys

sys.path.insert(0, "/opt/trn_rl_repo")

import numpy as np
import ml_dtypes

import concourse.bass as bass
import concourse.tile as tile
from concourse import mybir
from concourse.mybir import AluOpType as alu

AF = mybir.ActivationFunctionType
F32 = mybir.dt.float32
F32R = mybir.dt.float32r
BF16 = mybir.dt.bfloat16
BF = ml_dtypes.bfloat16

# problem dims (hardcoded; harness always calls with these shapes)
B, L, H = 16, 8192, 256
TE = 8
RIN = 10
OUT = 2 * H + TE  # 520
HH = 128
N_CORES = 8
SPC = B // N_CORES  # samples per core = 2
T = 512            # time tile
NT = L // T        # 16 tiles

E5 = float(np.exp(np.float32(5.0)))
EPS = 1e-5
DEBUG_DUMP = False

# ---------------------------------------------------------------------------
# custom DVE ops: registered into the concourse dve_ops registry at import.
# ---------------------------------------------------------------------------


def _register_gate_ops():
    import concourse.dve_ops as dve_ops
    from concourse.dve_spec import Spec, Src0, Src1, C0, C1, C2, select, lower
    from concourse.dve_spec import _has_src1
    from concourse.dve_uop import DveOpSpec

    if "GATE_G_ANT" in dve_ops._SUB_OPCODE_FOR_NAME:
        by_name = {op.name: op for op in dve_ops.OPS}
        return by_name["GATE_G_ANT"], by_name["GATE_B_ANT"]

    _y = Src0 + C0
    g_spec = Spec(
        body=select(_y >= C1, _y, Src1 * C2),
        reference=lambda in0, in1, s0, s1, imm2: np.where(
            (in0.astype(np.float32) + s0) >= s1,
            in0.astype(np.float32) + s0,
            in1.astype(np.float32) * imm2,
        ).astype(np.float32),
    )
    b_spec = Spec(
        body=(C0 - Src0) * Src1,
        reference=lambda in0, in1, s0, s1, imm2: (
            (s0 - in0.astype(np.float32)) * in1.astype(np.float32)
        ).astype(np.float32),
    )

    made = []
    for name, spec in (("GATE_G_ANT", g_spec), ("GATE_B_ANT", b_spec)):
        row = dve_ops._CUSTOM_DVE_ROW_BASE + len(dve_ops.OPS)
        shas = {}
        for ver in ("v3", "v4"):
            s = DveOpSpec(
                name=name, opcode=row, uops=lower(spec, ver=ver),
                rd1_en=_has_src1(spec),
            )
            shas[ver] = s.sha(ver)
        op = dve_ops.DveOp(name=name, spec=spec, subdim=False, uops_sha=shas)
        dve_ops.OPS.append(op)
        dve_ops._SUB_OPCODE_FOR_NAME[name] = row
        dve_ops.CUSTOM_DVE_SPECS[name] = spec
        made.append(op)
    return made[0], made[1]


GATE_G_OP, GATE_B_OP = _register_gate_ops()

# fp32 const blob layout: name -> (partitions, col offset, width)
BLOBF_LAYOUT = {
    "nckf": (128, 0, 2), "chf": (128, 2, 2), "chpf": (128, 4, 2),
    "nckb": (128, 6, 2), "chb": (128, 8, 2), "chpb": (128, 10, 2),
    "b1p": (HH, 12, 1), "eps16": (16, 13, 1),
    "half": (128, 14, 1),
}
BLOBF_W = 15
# bf16 const blob layout
BLOBB_LAYOUT = {
    "w1c0": (128, 0, HH), "w1c1": (128, 128, HH), "w1c2": (128, 256, HH),
    "w1c3": (128, 384, HH), "w1cte": (TE, 512, HH),
    "augw": (1, 640, HH),
    "indones": (128, 768, 16 * NT),   # stationary j: cols j*16..j*16+15, col j ones
    "w2ind": (128, 1024, 16 * NT),    # stationary j: col j = gh_w2 vector
    "onesr": (1, 1280, 128),
    "indA": (128, 1408, 64), "indB": (128, 1472, 64),
}
BLOBB_W = 1536


def build_core_program():
    """Build the per-core Bass program (2 samples, sequential)."""
    nc = bass.Bass()

    rnn_d = nc.dram_tensor("rnn", [SPC, 22, L], BF16, kind="ExternalInput")
    st_d = nc.dram_tensor("st", [SPC, 8, 4 * T], BF16, kind="ExternalInput")
    blobw_d = nc.dram_tensor("blobw", [32, 1024], BF16, kind="ExternalInput")
    blobf_d = nc.dram_tensor("blobf", [128, BLOBF_W], F32, kind="ExternalInput")
    blobb_d = nc.dram_tensor("blobb", [128, BLOBB_W], BF16, kind="ExternalInput")
    y_d = nc.dram_tensor("y", [SPC, NT, T], F32, kind="ExternalOutput")
    dbg = {}
    if DEBUG_DUMP:
        for s in range(SPC):
            for nm in ("hf0", "hf1", "hb0", "hb1"):
                dbg[f"{nm}_s{s}"] = nc.dram_tensor(f"dbg_{nm}_s{s}", [128, L],
                                                   BF16, kind="ExternalOutput")
            dbg[f"mun_s{s}"] = nc.dram_tensor(f"dbg_mun_s{s}", [NT, T], BF16,
                                              kind="ExternalOutput")
            dbg[f"r16_s{s}"] = nc.dram_tensor(f"dbg_r16_s{s}", [NT, T], F32,
                                              kind="ExternalOutput")

    with tile.TileContext(nc) as tc:
        _emit(tc, dict(rnn=rnn_d, st=st_d, blobf=blobf_d,
                       blobw=blobw_d, blobb=blobb_d, y=y_d, dbg=dbg))
    return _split_sync_waits(nc)


def _emit(tc, d):
    nc = tc.nc
    with tc.tile_pool(name="const", bufs=1) as const:
        blobf = const.tile([128, BLOBF_W], F32, tag="blobf", name="blobf")
        nc.sync.dma_start(blobf[:], d["blobf"][:])
        blobb = const.tile([128, BLOBB_W], BF16, tag="blobb", name="blobb")
        nc.sync.dma_start(blobb[:], d["blobb"][:])
        blobw = const.tile([32, 1024], BF16, tag="blobw", name="blobw")
        nc.sync.dma_start(blobw[:], d["blobw"][:])

        def cs(name):
            p, off, w = BLOBF_LAYOUT[name]
            return blobf[0:p, off:off + w]

        def csb(name):
            p, off, w = BLOBB_LAYOUT[name]
            return blobb[0:p, off:off + w]

        c = dict(
            wkf=blobw[:, 0:256], whf=blobw[:, 256:512],
            wkb=blobw[:, 512:768], whb=blobw[:, 768:1024],
            half=cs("half"),
            b1p=cs("b1p"), eps16=cs("eps16"),
            w1chunks=[csb("w1c0"), csb("w1c1"), csb("w1c2"), csb("w1c3"),
                      csb("w1cte")],
            augw=csb("augw"), indones=csb("indones"), w2ind=csb("w2ind"),
            onesr=csb("onesr"), indA=csb("indA"), indB=csb("indB"),
        )

        with tc.tile_pool(name="work", bufs=2) as work, \
             tc.tile_pool(name="sbuf", bufs=1) as sbuf, \
             tc.tile_pool(name="wps", bufs=2, space="PSUM") as wps, \
             tc.tile_pool(name="acc", bufs=1, space="PSUM") as accp:
            # warmup: one PE touch of each const blob so later matmuls carry
            # at most one new semaphore wait (the LW slot fits only one).
            wu = accp.tile([128, T], F32, tag="acc2", name="acc2w")[32:34, 0:2]
            nc.tensor.matmul(wu, blobb[0:1, 0:2], blobb[0:1, 0:2],
                             start=True, stop=False, skip_group_check=True)
            nc.tensor.matmul(wu, blobf[0:1, 0:1].bitcast(BF16),
                             blobf[0:1, 0:1].bitcast(BF16),
                             start=False, stop=False, skip_group_check=True)
            nc.tensor.matmul(wu, blobw[0:1, 0:2], blobw[0:1, 0:2],
                             start=False, stop=True, skip_group_check=True)
            # ACT/DVE queue warmups: wait each const-blob DMA semaphore once,
            # alone (compute instructions cannot mix a DMA wait with others).
            wsc = work.tile([1, 1], F32, tag="wsc", name="wsc", bufs=1)
            nc.scalar.copy(wsc[:], blobf[0:1, 0:1])
            wsv = work.tile([1, 1], F32, tag="wsv", name="wsv", bufs=1)
            nc.vector.tensor_scalar(wsv[:], blobf[0:1, 0:1], 0.0, None, alu.add)
            # sample tiles are shared between the two (sequential) samples:
            # the second sample's DMAs/scans overwrite them, so its matmuls
            # wait on a single producer semaphore instead of released-pool
            # overlap dependencies.
            tiles = dict(
                hfw=sbuf.tile([128, 2 * L], BF16, tag="hfw", name="hfw"),
                hbw=sbuf.tile([128, 2 * L], BF16, tag="hbw", name="hbw"),
                rnn=[sbuf.tile([32, L], BF16, tag=f"rnn{p}",
                               name=f"rnn{p}") for p in (0, 1)],
                st16=sbuf.tile([8, 4 * T], BF16, tag="st16", name="st16"),
                mu_row=sbuf.tile([1, L], BF16, tag="mu_row", name="mu_row"),
                r_row=sbuf.tile([1, L], BF16, tag="r_row", name="r_row"),
                acc1=accp.tile([128, T], F32, tag="acc1", name="acc1"),
                acc2=accp.tile([128, T], F32, tag="acc2", name="acc2"),
            )
            for s in range(SPC):
                cx = _emit_scan_phase(tc, d, s, c, work, wps, accp, tiles)
                _emit_finalize(tc, d, s, c, work, wps, accp, tiles, cx)
                _emit_head(tc, d, s, c, work, wps, accp, tiles, cx)


def _gate_pair(nc, work, wps, c, direction, rnn_mov, hw, lo, init0, init1,
               rev):
    """Both chunks of one (direction, tile): wide [128, 2T] pipeline.

    The matmul contracts 32 rows: [rnn_hi; rnn_lo; rnn_hi; 1; 1] against
    [W_hi; W_hi; W_lo; c_hi; c_lo], so k/v arrive in PSUM fully biased at
    fp32-grade accuracy.  hw is the combined h tile [128, 2L]; chunk c's
    output range is [c*L + lo, c*L + lo + T).
    """
    wk = c["wkf" if direction == "f" else "wkb"]
    wh = c["whf" if direction == "f" else "whb"]

    k_ps = wps.tile([128, 2 * T], F32, tag="k_ps", name="k_ps", bufs=2)
    v_ps = wps.tile([128, 2 * T], F32, tag="v_ps", name="v_ps", bufs=1)
    for ch in (0, 1):
        csl = slice(ch * 128, (ch + 1) * 128)
        tsl = slice(ch * T, (ch + 1) * T)
        nc.tensor.matmul(k_ps[:, tsl], wk[:, csl], rnn_mov, start=True,
                         stop=True)
        nc.tensor.matmul(v_ps[:, tsl], wh[:, csl], rnn_mov, start=True,
                         stop=True)
    # sgk = sigmoid(k): relative accuracy where small is what 1-a needs
    sgk = work.tile([128, 2 * T], BF16, tag="sgk", name="sgk")
    nc.scalar.activation(sgk[:], k_ps[:], AF.Sigmoid)
    # a = 1 - sgk in fp32 on Pool
    a = work.tile([128, 2 * T], F32, tag="a", name="a")
    nc.gpsimd.tensor_scalar(a[:], sgk[:], -1.0, 1.0, alu.mult, alu.add)
    # vp = v + 0.5 in fp32 first (the Pool-side branch predicate is on the
    # critical path into copy_predicated), then sgm.
    vp = work.tile([128, 2 * T], F32, tag="vp", name="vp", bufs=3)
    nc.scalar.activation(vp[:], v_ps[:], AF.Identity,
                         bias=c["half"][:, 0:1])
    # mge = (v >= 0) == (vp >= 0.5), exact in fp32, on Pool
    mge = work.tile([128, 2 * T], mybir.dt.uint8, tag="mge", name="mge", bufs=3)
    nc.gpsimd.tensor_scalar(mge[:], vp[:], 0.5, None, alu.is_ge)
    sgm = work.tile([128, 2 * T], BF16, tag="sgm", name="sgm", bufs=3)
    nc.scalar.activation(sgm[:], v_ps[:], AF.Sigmoid)
    # g = mge ? vp : e5*sgm
    g = work.tile([128, 2 * T], BF16, tag="g", name="g", bufs=3)
    nc.vector.tensor_scalar(g[:], sgm[:], E5, None, alu.mult)
    nc.vector.copy_predicated(g[:], mge[:], vp[:])
    # b = sigmoid(k) * g
    b = work.tile([128, 2 * T], BF16, tag="b", name="b")
    nc.vector.tensor_tensor(b[:], sgk[:], g[:], alu.mult)
    for ch, init in ((0, init0), (1, init1)):
        tsl = slice(ch * T, (ch + 1) * T)
        out_ap = hw[:, ch * L + lo:ch * L + lo + T]
        if rev:
            nc.vector.tensor_tensor_scan(out_ap[:, ::-1], a[:, tsl][:, ::-1],
                                         b[:, tsl][:, ::-1], init,
                                         alu.mult, alu.add)
        else:
            nc.vector.tensor_tensor_scan(out_ap, a[:, tsl], b[:, tsl], init,
                                         alu.mult, alu.add)


def _emit_scan_phase(tc, d, s, c, work, wps, accp, tiles):
    nc = tc.nc
    if True:
        hfw = tiles["hfw"]
        hbw = tiles["hbw"]
        rnn = tiles["rnn"][s % 2]
        nc.sync.dma_start(rnn[0:22, :], d["rnn"][s])
        nc.sync.dma_start(rnn[22:32, :], rnn[0:10, :])
        st16 = tiles["st16"]
        nc.sync.dma_start(st16[:], d["st"][s])
        wst = work.tile([1, 1], BF16, tag="wst", name=f"wst_s{s}", bufs=1)
        nc.vector.tensor_scalar(wst[:], st16[0:1, 0:1], 0.0, None, alu.add)
        mu_row = tiles["mu_row"]
        r_row = tiles["r_row"]
        # two shared PSUM accumulator banks (regions at legal base
        # partitions): bank1 s1A@0:8 s1B@32:40 s2A@64:72;
        # bank2 s2B@0:8 y@32:48 (the warmup scratch overlays y pre-reset).
        acc1 = tiles["acc1"]
        acc2 = tiles["acc2"]
        regs = dict(s1A=acc1[0:8, :], s1B=acc1[32:40, :], s2A=acc1[64:72, :],
                    s2B=acc2[0:8, :], y=acc2[32:48, :])

        def hsl(hw, ch, lo_, w=T):
            return hw[:, ch * L + lo_:ch * L + lo_ + w]

        cx = dict(hfw=hfw, hbw=hbw, rnn=rnn, mu_row=mu_row, r_row=r_row,
                  st16=st16, hsl=hsl, **regs)

        # ---------------- pass 1: forward gates + scans (DVE) --------------
        for j in range(NT):
            sl = slice(j * T, (j + 1) * T)
            inits = [0.5 if j == 0 else hsl(hfw, ch, j * T - 1, 1)
                     for ch in (0, 1)]
            _gate_pair(nc, work, wps, c, "f", rnn[:, sl], hfw, j * T,
                       inits[0], inits[1], rev=False)

        # ---- pass 2: backward gates + scans + split stats; once the upper
        # half of the LN statistics is closed (jj == 7), finalize it and
        # interleave the upper-half MLP head with the rest of pass 2. ----
        for jj in range(NT):
            tj = NT - 1 - jj
            lo, hi = tj * T, (tj + 1) * T
            binits = [0.5 if jj == 0 else hsl(hbw, ch, hi, 1)
                      for ch in (0, 1)]
            _gate_pair(nc, work, wps, c, "b", rnn[:, lo:hi], hbw, lo,
                       binits[0], binits[1], rev=True)

            half = "A" if tj >= 8 else "B"
            r8 = tj - 8 if tj >= 8 else tj
            ind = c["indA" if half == "A" else "indB"][:, r8 * 8:(r8 + 1) * 8]
            s1r = regs["s1" + half]
            s2r = regs["s2" + half]
            first = (jj == 0) or (jj == 8)
            Xs = [hsl(hfw, 0, lo), hsl(hfw, 1, lo),
                  hsl(hbw, 0, lo), hsl(hbw, 1, lo)]
            for i4, xt in enumerate(Xs):
                nc.tensor.matmul(s1r, ind, xt,
                                 start=(first and i4 == 0),
                                 stop=(jj in (7, 15) and i4 == 3),
                                 skip_group_check=True)
            for i4, hw in enumerate((hfw, hbw)):
                src0 = hw[:, :].rearrange("p (c l) -> p c l", l=L)[:, :,
                                                                  lo:lo + T]
                sq = work.tile([128, 2 * T], BF16, tag=f"sq{i4}", name="sq", bufs=1)
                sqv = sq[:, :].rearrange("p (c t) -> p c t", t=T)
                nc.vector.tensor_tensor(sqv, src0, src0, alu.mult)
                for ch in (0, 1):
                    nc.tensor.matmul(s2r, ind, sq[:, ch * T:(ch + 1) * T],
                                     start=(first and i4 == 0 and ch == 0),
                                     stop=(jj in (7, 15) and i4 == 1
                                           and ch == 1),
                                     skip_group_check=True)
            if jj == 7:
                _emit_finalize_half(tc, d, s, c, work, tiles, cx, "A")
            if jj in (9, 11, 13, 15):
                _emit_head_pair(tc, d, s, c, work, wps, tiles, cx, jj - 1,
                                first_pair=(jj == 9))
        return cx


def _emit_finalize_half(tc, d, s, c, work, tiles, cx, half):
    """LN stats -> -mu and rsqrt(var+eps) for one 8-tile half.

    half A covers tiles 8..15 (positions [8T, 16T)), B covers 0..7.
    st16 is [8, 4T]: cols [0:T]=-s1te/OUT lower, [T:2T] upper,
    [2T:3T]=s2te/OUT lower, [3T:4T] upper.
    """
    nc = tc.nc
    st16 = cx["st16"]; mu_row = cx["mu_row"]; r_row = cx["r_row"]
    s1r = cx["s1" + half]
    s2r = cx["s2" + half]
    c1 = T if half == "A" else 0
    mun = work.tile([8, T], BF16, tag="mun", name=f"mun{half}_s{s}", bufs=1)
    nc.vector.scalar_tensor_tensor(mun[:], s1r, -1.0 / OUT,
                                   st16[:, c1:c1 + T], alu.mult, alu.add)
    e2 = work.tile([8, T], F32, tag="e2h", name="e2h", bufs=1)
    nc.vector.scalar_tensor_tensor(e2[:], s2r, 1.0 / OUT,
                                   st16[:, 2 * T + c1:3 * T + c1],
                                   alu.mult, alu.add)
    mu2 = work.tile([8, T], F32, tag="mu2h", name="mu2h", bufs=1)
    nc.vector.tensor_tensor(mu2[:], mun[:], mun[:], alu.mult)
    varb = work.tile([8, T], F32, tag="varbh", name="varbh", bufs=1)
    nc.vector.tensor_tensor(varb[:], e2[:], mu2[:], alu.subtract)
    lnv = work.tile([8, T], F32, tag="e2h", name="lnvh", bufs=1)
    nc.scalar.activation(lnv[:], varb[:], AF.Ln, bias=c["eps16"][0:8, 0:1])
    r16 = work.tile([8, T], BF16, tag="r16h", name=f"r16{half}_s{s}", bufs=1)
    nc.scalar.activation(r16[:], lnv[:], AF.Exp, scale=-0.5)
    off = 8 * T if half == "A" else 0
    nc.sync.dma_start(
        mu_row[0:1, off:off + 8 * T].rearrange("p (j t) -> p j t", t=T),
        mun[:])
    nc.sync.dma_start(
        r_row[0:1, off:off + 8 * T].rearrange("p (j t) -> p j t", t=T),
        r16[:])
    wpr = work.tile([1, 1], BF16, tag="wpr", name=f"wpr{half}_s{s}", bufs=1)
    nc.gpsimd.tensor_scalar(wpr[:], r_row[0:1, off:off + 1], 0.0, None,
                            alu.add)


def _emit_head_pair(tc, d, s, c, work, wps, tiles, cx, j2, first_pair):
    """MLP head for tiles (j2, j2+1); y accumulates into the shared bank.
    Emission order is pairs 8,10,12,14 (interleaved with pass 2) then
    0,2,4,6 - start/stop flags follow that order."""
    nc = tc.nc
    hfw = cx["hfw"]; hbw = cx["hbw"]; rnn = cx["rnn"]
    mu_row = cx["mu_row"]; r_row = cx["r_row"]; y_acc = cx["y"]
    hsl = cx["hsl"]
    m_ps = wps.tile([128, 2 * T], F32, tag="k_ps", name="m_ps", bufs=2)
    r_ps = wps.tile([128, 2 * T], F32, tag="v_ps", name="r_ps", bufs=1)
    for dj in (0, 1):
        j = j2 + dj
        sl = slice(j * T, (j + 1) * T)
        psl = slice(dj * T, (dj + 1) * T)
        Xs = [hsl(hfw, 0, j * T), hsl(hfw, 1, j * T),
              hsl(hbw, 0, j * T), hsl(hbw, 1, j * T),
              rnn[0:TE, sl]]
        for i4, (wc, xt) in enumerate(zip(c["w1chunks"], Xs)):
            nc.tensor.matmul(m_ps[:, psl], wc, xt, start=(i4 == 0),
                             stop=False)
        nc.tensor.matmul(m_ps[:, psl], c["augw"], mu_row[0:1, sl],
                         start=False, stop=True)
        nc.tensor.matmul(r_ps[:, psl], c["onesr"], r_row[0:1, sl],
                         start=True, stop=True)
    rb = work.tile([128, 2 * T], BF16, tag="rb", name="rb")
    nc.scalar.copy(rb[:], r_ps[:])
    zr = work.tile([128, 2 * T], BF16, tag="zr", name="zr")
    nc.vector.tensor_tensor(zr[:], m_ps[:], rb[:], alu.mult)
    gel = work.tile([128, 2 * T], BF16, tag="gel", name="gel")
    nc.scalar.activation(gel[:], zr[:], AF.Gelu_apprx_tanh,
                         bias=c["b1p"][:, 0:1])
    for dj in (0, 1):
        j = j2 + dj
        nc.tensor.matmul(y_acc, c["w2ind"][:, j * 16:(j + 1) * 16],
                         gel[:, dj * T:(dj + 1) * T],
                         start=(first_pair and dj == 0),
                         stop=(j2 == 6 and dj == 1),
                         skip_group_check=True)


def _emit_finalize(tc, d, s, c, work, wps, accp, tiles, cx):
    nc = tc.nc
    _emit_finalize_half(tc, d, s, c, work, tiles, cx, "B")
    if DEBUG_DUMP:
        dbg = d["dbg"]
        hfw = cx["hfw"]; hbw = cx["hbw"]
        for nm, hw, ch in (("hf0", hfw, 0), ("hf1", hfw, 1),
                           ("hb0", hbw, 0), ("hb1", hbw, 1)):
            nc.sync.dma_start(dbg[f"{nm}_s{s}"][:],
                              hw[:, ch * L:(ch + 1) * L])


def _emit_head(tc, d, s, c, work, wps, accp, tiles, cx):
    nc = tc.nc
    for j2 in (0, 2, 4, 6):
        _emit_head_pair(tc, d, s, c, work, wps, tiles, cx, j2,
                        first_pair=False)
    y_sb = work.tile([NT, T], F32, tag="y_sb2", name="y_sb", bufs=1)
    nc.scalar.copy(y_sb[:], cx["y"])
    nc.sync.dma_start(d["y"][s], y_sb[:])


def _split_sync_waits(nc):
    """Rewrite the module so no instruction carries more than one sync wait
    or update: this walrus build can embed only a single semaphore op per
    ISA struct.  Extra waits move to single-wait NoOps inserted just before
    the instruction on the same engine queue; extra updates move to a NoOp
    just after it."""
    n = [0]

    def mknop(engine, waits, updates):
        n[0] += 1
        nop = mybir.InstNoOp(name=f"I-SW{n[0]}", ins=[], outs=[])
        nop.engine = engine
        nop.sync_info = mybir.SyncInfo(on_wait=waits, on_update=updates)
        return nop

    for f in nc.m.functions:
        for blk in f.blocks:
            out = []
            for inst in blk.instructions:
                si = inst.sync_info
                if si is None:
                    out.append(inst)
                    continue
                waits = list(si.on_wait or [])
                updates = list(si.on_update or [])
                while len(waits) > 1:
                    out.append(mknop(inst.engine, [waits.pop(0)], []))
                post = []
                # DMA completion updates fire from hardware; never move them.
                while inst.opcode != "DMACopy" and len(updates) > 1:
                    post.append(mknop(inst.engine, [], [updates.pop()]))
                inst.sync_info = mybir.SyncInfo(on_wait=waits, on_update=updates)
                out.append(inst)
                out.extend(post)
            blk.instructions = out
    return nc


_CACHED_NC = None


def _get_nc():
    global _CACHED_NC
    if _CACHED_NC is None:
        _CACHED_NC = build_core_program()
    return _CACHED_NC


def host_prep(inputs):
    """Fold weights and the te-MLP on the host."""
    f32 = np.float32
    g = {k: np.asarray(v, dtype=f32) for k, v in inputs.items()}

    # te encoding from t (host): h1 = relu(tsh*w1+b1); te = h1@w2+b2
    tsh = ((g["t"] - g["t"][:, :1]) / f32(inputs["time_scale"])).astype(f32)
    h1 = np.maximum(tsh[..., None] * g["te_w1"][0] + g["te_b1"], 0.0)
    te = (h1 @ g["te_w2"] + g["te_b2"]).astype(f32)          # (B, L, 8)
    s1te = te.sum(-1).astype(f32)                             # (B, L)
    s2te = (te * te).sum(-1).astype(f32)

    xm = (g["x"] * g["mask"][..., None]).astype(f32)          # (B, L, 2)

    # device rnn row order: [te(0:8); x(8:10)]; reference rnn = [x(2); te(8)]
    perm = np.array([2, 3, 4, 5, 6, 7, 8, 9, 0, 1])

    def fold(proj_w, proj_b, wz, bz, wh, bh):
        Wk = (proj_w @ wz).astype(f32)[perm]
        ck = (proj_b @ wz + bz).astype(f32)
        Wh = (proj_w @ wh).astype(f32)[perm]
        chv = (proj_b @ wh + bh).astype(f32)
        return Wk, ck, Wh, chv

    Wkf, ckf, Whf, chf = fold(g["fproj_w"], g["fproj_b"], g["f_wz"],
                              g["f_bz"], g["f_wh"], g["f_bh"])
    Wkb, ckb, Whb, chb = fold(g["bproj_w"], g["bproj_b"], g["b_wz"],
                              g["b_bz"], g["b_wh"], g["b_bh"])

    def cols(v):  # (256,) -> (128, 2), column c = chunk c
        return np.ascontiguousarray(v.reshape(2, 128).T)

    W1g = (g["ln_g"][:, None] * g["gh_w1"]).astype(f32)
    W1g_bf = W1g.astype(BF)
    colsum = W1g_bf.astype(f32).sum(0)
    b1p = (g["gh_b1"] + g["ln_b"] @ g["gh_w1"]).astype(f32)

    blobf = np.zeros((128, BLOBF_W), dtype=f32)
    Wcat = np.concatenate([Wkf, Whf, Wkb, Whb], axis=1).astype(f32)  # (10,1024)
    W_hi = Wcat.astype(BF).astype(f32)
    W_lo = (Wcat - W_hi).astype(BF)
    c_cat = np.concatenate([ckf, chf, ckb, chb]).astype(f32)         # (1024,)
    c_hi = c_cat.astype(BF).astype(f32)
    c_lo = (c_cat - c_hi).astype(BF)
    blobw = np.concatenate([W_hi.astype(BF), W_hi.astype(BF),
                            c_hi.astype(BF).reshape(1, -1),
                            c_lo.reshape(1, -1), W_lo], axis=0)       # (32,1024)

    def put(name, val):
        p, off, w = BLOBF_LAYOUT[name]
        assert val.shape == (p, w), (name, val.shape)
        blobf[0:p, off:off + w] = val

    put("half", np.full((128, 1), 0.5, dtype=f32))
    put("b1p", b1p.reshape(HH, 1))
    put("eps16", np.full((16, 1), EPS, dtype=f32))

    blobb = np.zeros((128, BLOBB_W), dtype=BF)

    def putb(name, val):
        p, off, w = BLOBB_LAYOUT[name]
        assert val.shape == (p, w), (name, val.shape)
        blobb[0:p, off:off + w] = val.astype(BF)

    for i in range(4):
        putb(f"w1c{i}", W1g_bf[i * 128:(i + 1) * 128, :])
    putb("w1cte", W1g_bf[512:520, :])
    putb("augw", colsum.reshape(1, HH))
    ind = np.zeros((128, 16 * NT), dtype=f32)
    w2i = np.zeros((128, 16 * NT), dtype=f32)
    for j in range(NT):
        ind[:, j * 16 + j] = 1.0
        w2i[:, j * 16 + j] = g["gh_w2"].reshape(-1)
    putb("indones", ind)
    putb("w2ind", w2i)
    putb("onesr", np.ones((1, 128), dtype=f32))
    iA = np.zeros((128, 64), dtype=f32)
    iB = np.zeros((128, 64), dtype=f32)
    for r in range(8):
        iA[:, r * 8 + r] = 1.0
        iB[:, r * 8 + r] = 1.0
    putb("indA", iA)
    putb("indB", iB)

    return dict(blobf=blobf, blobw=blobw, blobb=blobb), te, s1te, s2te, xm


def make_in_maps(inputs):
    wmap, te, s1te, s2te, xm = host_prep(inputs)
    f32 = np.float32
    rnn_f32 = np.concatenate([te, xm], axis=-1).astype(f32)   # (B, L, 10)
    rnn_hi = rnn_f32.astype(BF).astype(f32)
    rnn_lo = (rnn_f32 - rnn_hi).astype(BF)
    ones = np.ones((B, L, 1), np.float32)
    # rows: [hi(10); lo(10); 1; 1] sent over the wire; the device duplicates
    # hi into rows 22:32 to pair with blobw = [Whi; Wlo; c_hi; c_lo; Whi].
    rnn_all = np.concatenate([rnn_hi.astype(BF), rnn_lo,
                              ones.astype(BF), ones.astype(BF)],
                             axis=-1)                          # (B, L, 22)
    in_maps = []
    for i in range(N_CORES):
        m = dict(wmap)
        rnn_c = np.empty((SPC, 22, L), BF)
        st_c = np.empty((SPC, 8, 4 * T), BF)
        for s in range(SPC):
            bidx = i * SPC + s
            rnn_c[s] = rnn_all[bidx].T
            s1r = (-s1te[bidx] / OUT).reshape(NT, T)
            s2r = (s2te[bidx] / OUT).reshape(NT, T)
            st_c[s, :, 0:T] = s1r[0:8]
            st_c[s, :, T:2 * T] = s1r[8:16]
            st_c[s, :, 2 * T:3 * T] = s2r[0:8]
            st_c[s, :, 3 * T:4 * T] = s2r[8:16]
        m["rnn"] = np.ascontiguousarray(rnn_c)
        m["st"] = np.ascontiguousarray(st_c)
        in_maps.append(m)
    return in_maps


def _kernel_host(inputs):
    """Validated host fallback: same linear-recurrence formulation (numpy)."""
    f32 = np.float32
    g = {k: np.asarray(v, dtype=f32) for k, v in inputs.items()}

    def sig(z):
        out = np.exp(-np.abs(z))
        return np.where(z >= 0, 1.0 / (1.0 + out), out / (1.0 + out))

    xm = g["x"] * g["mask"][..., None]
    tshv = (g["t"] - g["t"][:, :1]) / g["time_scale"]
    h1 = np.maximum(tshv[..., None] * g["te_w1"][0] + g["te_b1"], 0.0)
    t_enc = (h1 @ g["te_w2"] + g["te_b2"]).astype(f32)
    rnn = np.concatenate([xm, t_enc], axis=-1)

    def scan(pw, pb, wz, bz, wh, bh, reverse):
        k = (rnn @ (pw @ wz) + (pb @ wz + bz)).astype(f32)
        v = (rnn @ (pw @ wh) + (pb @ wh + bh)).astype(f32)
        a = sig(-k)
        bv = sig(k) * np.where(v >= 0, v + 0.5, f32(np.exp(5.0)) * sig(v))
        if reverse:
            a = a[:, ::-1]; bv = bv[:, ::-1]
        h = np.empty_like(a)
        st = np.full((B, H), 0.5, dtype=f32)
        for i in range(L):
            st = a[:, i] * st + bv[:, i]
            h[:, i] = st
        return h[:, ::-1] if reverse else h

    hf = scan(g["fproj_w"], g["fproj_b"], g["f_wz"], g["f_bz"], g["f_wh"],
              g["f_bh"], False)
    hb = scan(g["bproj_w"], g["bproj_b"], g["b_wz"], g["b_bz"], g["b_wh"],
              g["b_bh"], True)
    X = np.concatenate([hf, hb, t_enc], axis=-1)
    mu = X.mean(-1, keepdims=True)
    var = ((X - mu) ** 2).mean(-1, keepdims=True)
    Xn = (X - mu) / np.sqrt(var + 1e-5) * g["ln_g"] + g["ln_b"]
    z = Xn @ g["gh_w1"] + g["gh_b1"]
    gel = 0.5 * z * (1.0 + np.tanh(f32(np.sqrt(2 / np.pi))
                                   * (z + f32(0.044715) * z ** 3)))
    return (gel @ g["gh_w2"] + g["gh_b2"]).astype(f32)




_RUNNER = None


def _get_runner():
    """Cached shard_map executable for the per-core program (the stock
    run_bass_kernel_spmd path rebuilds and retraces a fresh jax.jit on every
    call)."""
    global _RUNNER
    if _RUNNER is not None:
        return _RUNNER
    import jax
    import numpy as _np
    from jax.sharding import Mesh, PartitionSpec
    from jax.experimental.shard_map import shard_map
    from concourse import bass2jax, mybir as _mb
    from concourse.bass2jax import (_bass_exec_p, install_neuronx_cc_hook,
                                    partition_id_tensor)

    nc = _get_nc()
    install_neuronx_cc_hook()
    partition_name = (nc.partition_id_tensor.name
                      if nc.partition_id_tensor else None)
    in_names, out_names, out_avals, zero_outs = [], [], [], []
    for alloc in nc.m.functions[0].allocations:
        if not isinstance(alloc, _mb.MemoryLocationSet):
            continue
        name = alloc.memorylocations[0].name
        if alloc.kind == "ExternalInput":
            if name != partition_name:
                in_names.append(name)
        elif alloc.kind == "ExternalOutput":
            shape = tuple(alloc.tensor_shape)
            dtype = _mb.dt.np(alloc.dtype)
            out_names.append(name)
            out_avals.append(jax.core.ShapedArray(shape, dtype))
            zero_outs.append(_np.zeros(shape, dtype))
    n_params = len(in_names)
    n_outs = len(out_avals)
    in_names_all = list(in_names) + list(out_names)
    if partition_name is not None:
        in_names_all.append(partition_name)

    def _body(*args):
        operands = list(args)
        if partition_name is not None:
            operands.append(partition_id_tensor())
        outs = _bass_exec_p.bind(
            *operands, out_avals=tuple(out_avals), in_names=tuple(in_names_all),
            out_names=tuple(out_names), lowering_input_output_aliases=(),
            sim_require_finite=True, sim_require_nnan=True, nc=nc)
        return tuple(outs)

    devices = jax.devices()[:N_CORES]
    mesh = Mesh(_np.asarray(devices), ("core",))
    repl = {"blobf", "blobw", "blobb"}
    in_specs = tuple(PartitionSpec() if nm in repl else PartitionSpec("core")
                     for nm in in_names) + (PartitionSpec("core"),) * n_outs
    out_specs = (PartitionSpec("core"),) * n_outs
    sharded = jax.jit(
        shard_map(_body, mesh=mesh, in_specs=in_specs, out_specs=out_specs,
                  check_rep=False),
        donate_argnums=tuple(range(n_params, n_params + n_outs)),
        keep_unused=True)
    _RUNNER = (sharded, in_names, out_names, out_avals, zero_outs)
    return _RUNNER


def _run_cached(in_maps):
    import numpy as _np
    sharded, in_names, out_names, out_avals, zero_outs = _get_runner()
    repl = {"blobf", "blobw", "blobb"}
    concat_in = [_np.asarray(in_maps[0][nm]) if nm in repl else
                 _np.concatenate([_np.asarray(in_maps[c][nm])
                                  for c in range(N_CORES)], axis=0)
                 for nm in in_names]
    concat_zeros = [_np.zeros((N_CORES * z.shape[0], *z.shape[1:]), z.dtype)
                    for z in zero_outs]
    out_arrs = sharded(*concat_in, *concat_zeros)
    return [{nm: _np.asarray(out_arrs[i]).reshape(N_CORES,
                                                  *out_avals[i].shape)[c]
             for i, nm in enumerate(out_names)} for c in range(N_CORES)]


def kernel(**inputs) -> np.ndarray:
    in_maps = make_in_maps(inputs)
    results = _run_cached(in_maps)
    y = np.concatenate([results[i]["y"].reshape(SPC, L)
                        for i in range(N_CORES)], axis=0)
    b2 = np.float32(np.asarray(inputs["gh_b2"]).reshape(-1)[0])
    return (y + b2).reshape(B, L, 1).astype(np.float32)


if __name__ == "__main__":
    nc = build_core_program()
    print("built program")


# revision 68
# speedup vs baseline: 2.9056x; 2.1407x over previous
"""BiDirectionalMinGRU Trainium2 kernel.

Strategy
--------
Data-parallel over batch: 16 samples / 8 cores = 2 samples per core,
processed sequentially, weights replicated.  The minGRU log-space scan is
computed as the equivalent linear recurrence h_t = a_t*h_{t-1} + b_t with
a = 1 - sigmoid(k) kept in fp32 (20% of gates sit above 0.9999; bf16 or
the sigmoid(-k) table's absolute error would destroy the decay rates) and
b = sigmoid(k)*g(v) in bf16.

Key design points:
 - Gate projections run as ONE bf16 matmul per (k/v, chunk) with 32
   contraction rows [rnn_hi; rnn_lo; 1; 1; rnn_hi] against
   [W_hi; W_hi; c_hi; c_lo; W_lo]: fp32-grade accuracy at bf16 PE speed
   (1 cyc/col), with the gate biases folded in, so k/v arrive in PSUM
   fully biased and the wide [128, 2T] ACT/DVE/Pool ops need no
   per-chunk bias (f32r was tried first: its precision broke the
   saturated decay channels).
 - Engine placement per (direction, tile): PE matmuls; ACT sigmoid(k),
   v+0.5, sigmoid(v); Pool a = 1-sgk and the exact-fp32 branch predicate
   (v+0.5 >= 0.5); DVE e5-scale, copy_predicated select of g's branches,
   b-multiply, and both directions' scans (reversed APs for backward).
 - LN stats accumulate into PSUM via indicator-column stationaries, split
   into two 8-tile halves so the upper half of the MLP head overlaps the
   backward sweep; mean is folded into the head matmul as a (-mu)*colsum
   contraction row; the te encoding, its stats, and the +b2 bias are
   computed on the host (te depends only on input t).
 - This walrus build encodes at most ONE semaphore wait per instruction:
   _split_sync_waits() rewrites the module, moving extra waits onto
   single-wait NoOps, and "warmup" reads make each queue wait each DMA
   semaphore alone.  Custom DVE ops / partition_broadcast / Pool scans
   all fail codegen here and are not used.
"""

import sys

sys.path.insert(0, "/opt/trn_rl_repo")

import numpy as np
import ml_dtypes

import concourse.bass as bass
import concourse.tile as tile
from concourse import mybir
from concourse.mybir import AluOpType as alu

AF = mybir.ActivationFunctionType
F32 = mybir.dt.float32
F32R = mybir.dt.float32r
BF16 = mybir.dt.bfloat16
BF = ml_dtypes.bfloat16

# problem dims (hardcoded; harness always calls with these shapes)
B, L, H = 16, 8192, 256
TE = 8
RIN = 10
OUT = 2 * H + TE  # 520
HH = 128
N_CORES = 8
SPC = B // N_CORES  # samples per core = 2
T = 512            # time tile
NT = L // T        # 16 tiles

E5 = float(np.exp(np.float32(5.0)))
EPS = 1e-5
DEBUG_DUMP = False

# ---------------------------------------------------------------------------
# custom DVE ops: registered into the concourse dve_ops registry at import.
# ---------------------------------------------------------------------------


def _register_gate_ops():
    import concourse.dve_ops as dve_ops
    from concourse.dve_spec import Spec, Src0, Src1, C0, C1, C2, select, lower
    from concourse.dve_spec import _has_src1
    from concourse.dve_uop import DveOpSpec

    if "GATE_G_ANT" in dve_ops._SUB_OPCODE_FOR_NAME:
        by_name = {op.name: op for op in dve_ops.OPS}
        return by_name["GATE_G_ANT"], by_name["GATE_B_ANT"]

    _y = Src0 + C0
    g_spec = Spec(
        body=select(_y >= C1, _y, Src1 * C2),
        reference=lambda in0, in1, s0, s1, imm2: np.where(
            (in0.astype(np.float32) + s0) >= s1,
            in0.astype(np.float32) + s0,
            in1.astype(np.float32) * imm2,
        ).astype(np.float32),
    )
    b_spec = Spec(
        body=(C0 - Src0) * Src1,
        reference=lambda in0, in1, s0, s1, imm2: (
            (s0 - in0.astype(np.float32)) * in1.astype(np.float32)
        ).astype(np.float32),
    )

    made = []
    for name, spec in (("GATE_G_ANT", g_spec), ("GATE_B_ANT", b_spec)):
        row = dve_ops._CUSTOM_DVE_ROW_BASE + len(dve_ops.OPS)
        shas = {}
        for ver in ("v3", "v4"):
            s = DveOpSpec(
                name=name, opcode=row, uops=lower(spec, ver=ver),
                rd1_en=_has_src1(spec),
            )
            shas[ver] = s.sha(ver)
        op = dve_ops.DveOp(name=name, spec=spec, subdim=False, uops_sha=shas)
        dve_ops.OPS.append(op)
        dve_ops._SUB_OPCODE_FOR_NAME[name] = row
        dve_ops.CUSTOM_DVE_SPECS[name] = spec
        made.append(op)
    return made[0], made[1]


GATE_G_OP, GATE_B_OP = _register_gate_ops()

# fp32 const blob layout: name -> (partitions, col offset, width)
BLOBF_LAYOUT = {
    "nckf": (128, 0, 2), "chf": (128, 2, 2), "chpf": (128, 4, 2),
    "nckb": (128, 6, 2), "chb": (128, 8, 2), "chpb": (128, 10, 2),
    "b1p": (HH, 12, 1), "eps16": (16, 13, 1),
    "half": (128, 14, 1),
}
BLOBF_W = 15
# bf16 const blob layout
BLOBB_LAYOUT = {
    "w1c0": (128, 0, HH), "w1c1": (128, 128, HH), "w1c2": (128, 256, HH),
    "w1c3": (128, 384, HH), "w1cte": (TE, 512, HH),
    "augw": (1, 640, HH),
    "indones": (128, 768, 16 * NT),   # stationary j: cols j*16..j*16+15, col j ones
    "w2ind": (128, 1024, 16 * NT),    # stationary j: col j = gh_w2 vector
    "onesr": (1, 1280, 128),
    "indA": (128, 1408, 64), "indB": (128, 1472, 64),
}
BLOBB_W = 1536


def build_core_program():
    """Build the per-core Bass program (2 samples, sequential)."""
    nc = bass.Bass()

    rnn_d = nc.dram_tensor("rnn", [SPC, 22, L], BF16, kind="ExternalInput")
    st_d = nc.dram_tensor("st", [SPC, 8, 4 * T], BF16, kind="ExternalInput")
    blobw_d = nc.dram_tensor("blobw", [32, 1024], BF16, kind="ExternalInput")
    blobf_d = nc.dram_tensor("blobf", [128, BLOBF_W], F32, kind="ExternalInput")
    blobb_d = nc.dram_tensor("blobb", [128, BLOBB_W], BF16, kind="ExternalInput")
    y_d = nc.dram_tensor("y", [SPC, NT, T], F32, kind="ExternalOutput")
    dbg = {}
    if DEBUG_DUMP:
        for s in range(SPC):
            for nm in ("hf0", "hf1", "hb0", "hb1"):
                dbg[f"{nm}_s{s}"] = nc.dram_tensor(f"dbg_{nm}_s{s}", [128, L],
                                                   BF16, kind="ExternalOutput")
            dbg[f"mun_s{s}"] = nc.dram_tensor(f"dbg_mun_s{s}", [NT, T], BF16,
                                              kind="ExternalOutput")
            dbg[f"r16_s{s}"] = nc.dram_tensor(f"dbg_r16_s{s}", [NT, T], F32,
                                              kind="ExternalOutput")

    with tile.TileContext(nc) as tc:
        _emit(tc, dict(rnn=rnn_d, st=st_d, blobf=blobf_d,
                       blobw=blobw_d, blobb=blobb_d, y=y_d, dbg=dbg))
    return _split_sync_waits(nc)


def _emit(tc, d):
    nc = tc.nc
    with tc.tile_pool(name="const", bufs=1) as const:
        blobf = const.tile([128, BLOBF_W], F32, tag="blobf", name="blobf")
        nc.sync.dma_start(blobf[:], d["blobf"][:])
        blobb = const.tile([128, BLOBB_W], BF16, tag="blobb", name="blobb")
        nc.sync.dma_start(blobb[:], d["blobb"][:])
        blobw = const.tile([32, 1024], BF16, tag="blobw", name="blobw")
        nc.sync.dma_start(blobw[:], d["blobw"][:])

        def cs(name):
            p, off, w = BLOBF_LAYOUT[name]
            return blobf[0:p, off:off + w]

        def csb(name):
            p, off, w = BLOBB_LAYOUT[name]
            return blobb[0:p, off:off + w]

        c = dict(
            wkf=blobw[:, 0:256], whf=blobw[:, 256:512],
            wkb=blobw[:, 512:768], whb=blobw[:, 768:1024],
            half=cs("half"),
            b1p=cs("b1p"), eps16=cs("eps16"),
            w1chunks=[csb("w1c0"), csb("w1c1"), csb("w1c2"), csb("w1c3"),
                      csb("w1cte")],
            augw=csb("augw"), indones=csb("indones"), w2ind=csb("w2ind"),
            onesr=csb("onesr"), indA=csb("indA"), indB=csb("indB"),
        )

        with tc.tile_pool(name="work", bufs=2) as work, \
             tc.tile_pool(name="sbuf", bufs=1) as sbuf, \
             tc.tile_pool(name="wps", bufs=2, space="PSUM") as wps, \
             tc.tile_pool(name="acc", bufs=1, space="PSUM") as accp:
            # warmup: one PE touch of each const blob so later matmuls carry
            # at most one new semaphore wait (the LW slot fits only one).
            wu = accp.tile([128, T], F32, tag="acc2", name="acc2w")[32:34, 0:2]
            nc.tensor.matmul(wu, blobb[0:1, 0:2], blobb[0:1, 0:2],
                             start=True, stop=False, skip_group_check=True)
            nc.tensor.matmul(wu, blobf[0:1, 0:1].bitcast(BF16),
                             blobf[0:1, 0:1].bitcast(BF16),
                             start=False, stop=False, skip_group_check=True)
            nc.tensor.matmul(wu, blobw[0:1, 0:2], blobw[0:1, 0:2],
                             start=False, stop=True, skip_group_check=True)
            # ACT/DVE queue warmups: wait each const-blob DMA semaphore once,
            # alone (compute instructions cannot mix a DMA wait with others).
            wsc = work.tile([1, 1], F32, tag="wsc", name="wsc", bufs=1)
            nc.scalar.copy(wsc[:], blobf[0:1, 0:1])
            wsv = work.tile([1, 1], F32, tag="wsv", name="wsv", bufs=1)
            nc.vector.tensor_scalar(wsv[:], blobf[0:1, 0:1], 0.0, None, alu.add)
            # sample tiles are shared between the two (sequential) samples:
            # the second sample's DMAs/scans overwrite them, so its matmuls
            # wait on a single producer semaphore instead of released-pool
            # overlap dependencies.
            tiles = dict(
                hfw=sbuf.tile([128, 2 * L], BF16, tag="hfw", name="hfw"),
                hbw=sbuf.tile([128, 2 * L], BF16, tag="hbw", name="hbw"),
                rnn=[sbuf.tile([32, L], BF16, tag=f"rnn{p}",
                               name=f"rnn{p}") for p in (0, 1)],
                st16=sbuf.tile([8, 4 * T], BF16, tag="st16", name="st16"),
                mu_row=sbuf.tile([1, L], BF16, tag="mu_row", name="mu_row"),
                r_row=sbuf.tile([1, L], BF16, tag="r_row", name="r_row"),
                acc1=accp.tile([128, T], F32, tag="acc1", name="acc1"),
                acc2=accp.tile([128, T], F32, tag="acc2", name="acc2"),
            )
            for s in range(SPC):
                cx = _emit_scan_phase(tc, d, s, c, work, wps, accp, tiles)
                _emit_finalize(tc, d, s, c, work, wps, accp, tiles, cx)
                _emit_head(tc, d, s, c, work, wps, accp, tiles, cx)


def _gate_pair(nc, work, wps, c, direction, rnn_mov, hw, lo, init0, init1,
               rev):
    """Both chunks of one (direction, tile): wide [128, 2T] pipeline.

    The matmul contracts 32 rows: [rnn_hi; rnn_lo; rnn_hi; 1; 1] against
    [W_hi; W_hi; W_lo; c_hi; c_lo], so k/v arrive in PSUM fully biased at
    fp32-grade accuracy.  hw is the combined h tile [128, 2L]; chunk c's
    output range is [c*L + lo, c*L + lo + T).
    """
    wk = c["wkf" if direction == "f" else "wkb"]
    wh = c["whf" if direction == "f" else "whb"]

    k_ps = wps.tile([128, 2 * T], F32, tag="k_ps", name="k_ps", bufs=2)
    v_ps = wps.tile([128, 2 * T], F32, tag="v_ps", name="v_ps", bufs=1)
    for ch in (0, 1):
        csl = slice(ch * 128, (ch + 1) * 128)
        tsl = slice(ch * T, (ch + 1) * T)
        nc.tensor.matmul(k_ps[:, tsl], wk[:, csl], rnn_mov, start=True,
                         stop=True)
        nc.tensor.matmul(v_ps[:, tsl], wh[:, csl], rnn_mov, start=True,
                         stop=True)
    # sgk = sigmoid(k): relative accuracy where small is what 1-a needs
    sgk = work.tile([128, 2 * T], BF16, tag="sgk", name="sgk")
    nc.scalar.activation(sgk[:], k_ps[:], AF.Sigmoid)
    # vp = v + 0.5 in fp32 first (the Pool-side branch predicate is on the
    # critical path into copy_predicated), then sgm.
    vp = work.tile([128, 2 * T], F32, tag="vp", name="vp", bufs=3)
    nc.scalar.activation(vp[:], v_ps[:], AF.Identity,
                         bias=c["half"][:, 0:1])
    # mge = (v >= 0) == (vp >= 0.5), exact in fp32, on Pool, ahead of the
    # decay coefficient (copy_predicated needs mge sooner than the scans
    # need a)
    mge = work.tile([128, 2 * T], mybir.dt.uint8, tag="mge", name="mge", bufs=3)
    nc.gpsimd.tensor_scalar(mge[:], vp[:], 0.5, None, alu.is_ge)
    # a = 1 - sgk in fp32 on Pool
    a = work.tile([128, 2 * T], F32, tag="a", name="a")
    nc.gpsimd.tensor_scalar(a[:], sgk[:], -1.0, 1.0, alu.mult, alu.add)
    sgm = work.tile([128, 2 * T], BF16, tag="sgm", name="sgm", bufs=3)
    nc.scalar.activation(sgm[:], v_ps[:], AF.Sigmoid)
    # g = mge ? vp : e5*sgm
    g = work.tile([128, 2 * T], BF16, tag="g", name="g", bufs=3)
    nc.vector.tensor_scalar(g[:], sgm[:], E5, None, alu.mult)
    nc.vector.copy_predicated(g[:], mge[:], vp[:])
    # b = sigmoid(k) * g
    b = work.tile([128, 2 * T], BF16, tag="b", name="b")
    nc.vector.tensor_tensor(b[:], sgk[:], g[:], alu.mult)
    for ch, init in ((0, init0), (1, init1)):
        tsl = slice(ch * T, (ch + 1) * T)
        out_ap = hw[:, ch * L + lo:ch * L + lo + T]
        if rev:
            nc.vector.tensor_tensor_scan(out_ap[:, ::-1], a[:, tsl][:, ::-1],
                                         b[:, tsl][:, ::-1], init,
                                         alu.mult, alu.add)
        else:
            nc.vector.tensor_tensor_scan(out_ap, a[:, tsl], b[:, tsl], init,
                                         alu.mult, alu.add)


def _emit_scan_phase(tc, d, s, c, work, wps, accp, tiles):
    nc = tc.nc
    if True:
        hfw = tiles["hfw"]
        hbw = tiles["hbw"]
        rnn = tiles["rnn"][s % 2]
        # column-halved loads so tile-0 gates start after the first half;
        # the hi-row duplication (pairs with W_lo) chains off its own half.
        Lh = L // 2
        nc.sync.dma_start(rnn[0:22, 0:Lh], d["rnn"][s][:, 0:Lh])
        nc.sync.dma_start(rnn[22:32, 0:Lh], rnn[0:10, 0:Lh])
        nc.sync.dma_start(rnn[0:22, Lh:L], d["rnn"][s][:, Lh:L])
        nc.sync.dma_start(rnn[22:32, Lh:L], rnn[0:10, Lh:L])
        st16 = tiles["st16"]
        nc.sync.dma_start(st16[:], d["st"][s])
        wst = work.tile([1, 1], BF16, tag="wst", name=f"wst_s{s}", bufs=1)
        nc.vector.tensor_scalar(wst[:], st16[0:1, 0:1], 0.0, None, alu.add)
        mu_row = tiles["mu_row"]
        r_row = tiles["r_row"]
        # two shared PSUM accumulator banks (regions at legal base
        # partitions): bank1 s1A@0:8 s1B@32:40 s2A@64:72;
        # bank2 s2B@0:8 y@32:48 (the warmup scratch overlays y pre-reset).
        acc1 = tiles["acc1"]
        acc2 = tiles["acc2"]
        regs = dict(s1A=acc1[0:8, :], s1B=acc1[32:40, :], s2A=acc1[64:72, :],
                    s2B=acc2[0:8, :], y=acc2[32:48, :])

        def hsl(hw, ch, lo_, w=T):
            return hw[:, ch * L + lo_:ch * L + lo_ + w]

        cx = dict(hfw=hfw, hbw=hbw, rnn=rnn, mu_row=mu_row, r_row=r_row,
                  st16=st16, hsl=hsl, **regs)

        # ---------------- pass 1: forward gates + scans (DVE) --------------
        for j in range(NT):
            sl = slice(j * T, (j + 1) * T)
            inits = [0.5 if j == 0 else hsl(hfw, ch, j * T - 1, 1)
                     for ch in (0, 1)]
            _gate_pair(nc, work, wps, c, "f", rnn[:, sl], hfw, j * T,
                       inits[0], inits[1], rev=False)

        # ---- pass 2: backward gates + scans + split stats; once the upper
        # half of the LN statistics is closed (jj == 7), finalize it and
        # interleave the upper-half MLP head with the rest of pass 2. ----
        for jj in range(NT):
            tj = NT - 1 - jj
            lo, hi = tj * T, (tj + 1) * T
            binits = [0.5 if jj == 0 else hsl(hbw, ch, hi, 1)
                      for ch in (0, 1)]
            _gate_pair(nc, work, wps, c, "b", rnn[:, lo:hi], hbw, lo,
                       binits[0], binits[1], rev=True)

            half = "A" if tj >= 8 else "B"
            r8 = tj - 8 if tj >= 8 else tj
            ind = c["indA" if half == "A" else "indB"][:, r8 * 8:(r8 + 1) * 8]
            s1r = regs["s1" + half]
            s2r = regs["s2" + half]
            first = (jj == 0) or (jj == 8)
            Xs = [hsl(hfw, 0, lo), hsl(hfw, 1, lo),
                  hsl(hbw, 0, lo), hsl(hbw, 1, lo)]
            for i4, xt in enumerate(Xs):
                nc.tensor.matmul(s1r, ind, xt,
                                 start=(first and i4 == 0),
                                 stop=(jj in (7, 15) and i4 == 3),
                                 skip_group_check=True)
            for i4, hw in enumerate((hfw, hbw)):
                src0 = hw[:, :].rearrange("p (c l) -> p c l", l=L)[:, :,
                                                                  lo:lo + T]
                sq = work.tile([128, 2 * T], BF16, tag=f"sq{i4}", name="sq", bufs=1)
                sqv = sq[:, :].rearrange("p (c t) -> p c t", t=T)
                nc.vector.tensor_tensor(sqv, src0, src0, alu.mult)
                for ch in (0, 1):
                    nc.tensor.matmul(s2r, ind, sq[:, ch * T:(ch + 1) * T],
                                     start=(first and i4 == 0 and ch == 0),
                                     stop=(jj in (7, 15) and i4 == 1
                                           and ch == 1),
                                     skip_group_check=True)
            if jj == 7:
                _emit_finalize_half(tc, d, s, c, work, tiles, cx, "A")
            if jj in (9, 11, 13, 15):
                _emit_head_pair(tc, d, s, c, work, wps, tiles, cx, jj - 1,
                                first_pair=(jj == 9))
        return cx


def _emit_finalize_half(tc, d, s, c, work, tiles, cx, half):
    """LN stats -> -mu and rsqrt(var+eps) for one 8-tile half.

    half A covers tiles 8..15 (positions [8T, 16T)), B covers 0..7.
    st16 is [8, 4T]: cols [0:T]=-s1te/OUT lower, [T:2T] upper,
    [2T:3T]=s2te/OUT lower, [3T:4T] upper.
    """
    nc = tc.nc
    st16 = cx["st16"]; mu_row = cx["mu_row"]; r_row = cx["r_row"]
    s1r = cx["s1" + half]
    s2r = cx["s2" + half]
    c1 = T if half == "A" else 0
    mun = work.tile([8, T], BF16, tag="mun", name=f"mun{half}_s{s}", bufs=1)
    nc.vector.scalar_tensor_tensor(mun[:], s1r, -1.0 / OUT,
                                   st16[:, c1:c1 + T], alu.mult, alu.add)
    e2 = work.tile([8, T], F32, tag="e2h", name="e2h", bufs=1)
    nc.vector.scalar_tensor_tensor(e2[:], s2r, 1.0 / OUT,
                                   st16[:, 2 * T + c1:3 * T + c1],
                                   alu.mult, alu.add)
    mu2 = work.tile([8, T], F32, tag="mu2h", name="mu2h", bufs=1)
    nc.vector.tensor_tensor(mu2[:], mun[:], mun[:], alu.mult)
    varb = work.tile([8, T], F32, tag="varbh", name="varbh", bufs=1)
    nc.vector.tensor_tensor(varb[:], e2[:], mu2[:], alu.subtract)
    lnv = work.tile([8, T], F32, tag="e2h", name="lnvh", bufs=1)
    nc.scalar.activation(lnv[:], varb[:], AF.Ln, bias=c["eps16"][0:8, 0:1])
    r16 = work.tile([8, T], BF16, tag="r16h", name=f"r16{half}_s{s}", bufs=1)
    nc.scalar.activation(r16[:], lnv[:], AF.Exp, scale=-0.5)
    off = 8 * T if half == "A" else 0
    nc.sync.dma_start(
        mu_row[0:1, off:off + 8 * T].rearrange("p (j t) -> p j t", t=T),
        mun[:])
    nc.sync.dma_start(
        r_row[0:1, off:off + 8 * T].rearrange("p (j t) -> p j t", t=T),
        r16[:])
    wpr = work.tile([1, 1], BF16, tag="wpr", name=f"wpr{half}_s{s}", bufs=1)
    nc.gpsimd.tensor_scalar(wpr[:], r_row[0:1, off:off + 1], 0.0, None,
                            alu.add)


def _emit_head_pair(tc, d, s, c, work, wps, tiles, cx, j2, first_pair):
    """MLP head for tiles (j2, j2+1); y accumulates into the shared bank.
    Emission order is pairs 8,10,12,14 (interleaved with pass 2) then
    0,2,4,6 - start/stop flags follow that order."""
    nc = tc.nc
    hfw = cx["hfw"]; hbw = cx["hbw"]; rnn = cx["rnn"]
    mu_row = cx["mu_row"]; r_row = cx["r_row"]; y_acc = cx["y"]
    hsl = cx["hsl"]
    m_ps = wps.tile([128, 2 * T], F32, tag="k_ps", name="m_ps", bufs=2)
    r_ps = wps.tile([128, 2 * T], F32, tag="v_ps", name="r_ps", bufs=1)
    for dj in (0, 1):
        j = j2 + dj
        sl = slice(j * T, (j + 1) * T)
        psl = slice(dj * T, (dj + 1) * T)
        Xs = [hsl(hfw, 0, j * T), hsl(hfw, 1, j * T),
              hsl(hbw, 0, j * T), hsl(hbw, 1, j * T),
              rnn[0:TE, sl]]
        for i4, (wc, xt) in enumerate(zip(c["w1chunks"], Xs)):
            nc.tensor.matmul(m_ps[:, psl], wc, xt, start=(i4 == 0),
                             stop=False)
        nc.tensor.matmul(m_ps[:, psl], c["augw"], mu_row[0:1, sl],
                         start=False, stop=True)
        nc.tensor.matmul(r_ps[:, psl], c["onesr"], r_row[0:1, sl],
                         start=True, stop=True)
    rb = work.tile([128, 2 * T], BF16, tag="rb", name="rb")
    nc.scalar.copy(rb[:], r_ps[:])
    zr = work.tile([128, 2 * T], BF16, tag="zr", name="zr")
    nc.vector.tensor_tensor(zr[:], m_ps[:], rb[:], alu.mult)
    gel = work.tile([128, 2 * T], BF16, tag="gel", name="gel")
    nc.scalar.activation(gel[:], zr[:], AF.Gelu_apprx_tanh,
                         bias=c["b1p"][:, 0:1])
    for dj in (0, 1):
        j = j2 + dj
        nc.tensor.matmul(y_acc, c["w2ind"][:, j * 16:(j + 1) * 16],
                         gel[:, dj * T:(dj + 1) * T],
                         start=(first_pair and dj == 0),
                         stop=(j2 == 6 and dj == 1),
                         skip_group_check=True)


def _emit_finalize(tc, d, s, c, work, wps, accp, tiles, cx):
    nc = tc.nc
    _emit_finalize_half(tc, d, s, c, work, tiles, cx, "B")
    if DEBUG_DUMP:
        dbg = d["dbg"]
        hfw = cx["hfw"]; hbw = cx["hbw"]
        for nm, hw, ch in (("hf0", hfw, 0), ("hf1", hfw, 1),
                           ("hb0", hbw, 0), ("hb1", hbw, 1)):
            nc.sync.dma_start(dbg[f"{nm}_s{s}"][:],
                              hw[:, ch * L:(ch + 1) * L])


def _emit_head(tc, d, s, c, work, wps, accp, tiles, cx):
    nc = tc.nc
    for j2 in (0, 2, 4, 6):
        _emit_head_pair(tc, d, s, c, work, wps, tiles, cx, j2,
                        first_pair=False)
    y_sb = work.tile([NT, T], F32, tag="y_sb2", name="y_sb", bufs=1)
    nc.scalar.copy(y_sb[:], cx["y"])
    nc.sync.dma_start(d["y"][s], y_sb[:])


def _split_sync_waits(nc):
    """Rewrite the module so no instruction carries more than one sync wait
    or update: this walrus build can embed only a single semaphore op per
    ISA struct.  Extra waits move to single-wait NoOps inserted just before
    the instruction on the same engine queue; extra updates move to a NoOp
    just after it."""
    n = [0]

    def mknop(engine, waits, updates):
        n[0] += 1
        nop = mybir.InstNoOp(name=f"I-SW{n[0]}", ins=[], outs=[])
        nop.engine = engine
        nop.sync_info = mybir.SyncInfo(on_wait=waits, on_update=updates)
        return nop

    for f in nc.m.functions:
        for blk in f.blocks:
            out = []
            for inst in blk.instructions:
                si = inst.sync_info
                if si is None:
                    out.append(inst)
                    continue
                waits = list(si.on_wait or [])
                updates = list(si.on_update or [])
                while len(waits) > 1:
                    out.append(mknop(inst.engine, [waits.pop(0)], []))
                post = []
                # DMA completion updates fire from hardware; never move them.
                while inst.opcode != "DMACopy" and len(updates) > 1:
                    post.append(mknop(inst.engine, [], [updates.pop()]))
                inst.sync_info = mybir.SyncInfo(on_wait=waits, on_update=updates)
                out.append(inst)
                out.extend(post)
            blk.instructions = out
    return nc


_CACHED_NC = None


def _get_nc():
    global _CACHED_NC
    if _CACHED_NC is None:
        _CACHED_NC = build_core_program()
    return _CACHED_NC


def host_prep(inputs):
    """Fold weights and the te-MLP on the host."""
    f32 = np.float32
    g = {k: np.asarray(v, dtype=f32) for k, v in inputs.items()}

    # te encoding from t (host): h1 = relu(tsh*w1+b1); te = h1@w2+b2
    tsh = ((g["t"] - g["t"][:, :1]) / f32(inputs["time_scale"])).astype(f32)
    h1 = np.maximum(tsh[..., None] * g["te_w1"][0] + g["te_b1"], 0.0)
    te = (h1 @ g["te_w2"] + g["te_b2"]).astype(f32)          # (B, L, 8)
    s1te = te.sum(-1).astype(f32)                             # (B, L)
    s2te = (te * te).sum(-1).astype(f32)

    xm = (g["x"] * g["mask"][..., None]).astype(f32)          # (B, L, 2)

    # device rnn row order: [te(0:8); x(8:10)]; reference rnn = [x(2); te(8)]
    perm = np.array([2, 3, 4, 5, 6, 7, 8, 9, 0, 1])

    def fold(proj_w, proj_b, wz, bz, wh, bh):
        Wk = (proj_w @ wz).astype(f32)[perm]
        ck = (proj_b @ wz + bz).astype(f32)
        Wh = (proj_w @ wh).astype(f32)[perm]
        chv = (proj_b @ wh + bh).astype(f32)
        return Wk, ck, Wh, chv

    Wkf, ckf, Whf, chf = fold(g["fproj_w"], g["fproj_b"], g["f_wz"],
                              g["f_bz"], g["f_wh"], g["f_bh"])
    Wkb, ckb, Whb, chb = fold(g["bproj_w"], g["bproj_b"], g["b_wz"],
                              g["b_bz"], g["b_wh"], g["b_bh"])

    def cols(v):  # (256,) -> (128, 2), column c = chunk c
        return np.ascontiguousarray(v.reshape(2, 128).T)

    W1g = (g["ln_g"][:, None] * g["gh_w1"]).astype(f32)
    W1g_bf = W1g.astype(BF)
    colsum = W1g_bf.astype(f32).sum(0)
    b1p = (g["gh_b1"] + g["ln_b"] @ g["gh_w1"]).astype(f32)

    blobf = np.zeros((128, BLOBF_W), dtype=f32)
    Wcat = np.concatenate([Wkf, Whf, Wkb, Whb], axis=1).astype(f32)  # (10,1024)
    W_hi = Wcat.astype(BF).astype(f32)
    W_lo = (Wcat - W_hi).astype(BF)
    c_cat = np.concatenate([ckf, chf, ckb, chb]).astype(f32)         # (1024,)
    c_hi = c_cat.astype(BF).astype(f32)
    c_lo = (c_cat - c_hi).astype(BF)
    blobw = np.concatenate([W_hi.astype(BF), W_hi.astype(BF),
                            c_hi.astype(BF).reshape(1, -1),
                            c_lo.reshape(1, -1), W_lo], axis=0)       # (32,1024)

    def put(name, val):
        p, off, w = BLOBF_LAYOUT[name]
        assert val.shape == (p, w), (name, val.shape)
        blobf[0:p, off:off + w] = val

    put("half", np.full((128, 1), 0.5, dtype=f32))
    put("b1p", b1p.reshape(HH, 1))
    put("eps16", np.full((16, 1), EPS, dtype=f32))

    blobb = np.zeros((128, BLOBB_W), dtype=BF)

    def putb(name, val):
        p, off, w = BLOBB_LAYOUT[name]
        assert val.shape == (p, w), (name, val.shape)
        blobb[0:p, off:off + w] = val.astype(BF)

    for i in range(4):
        putb(f"w1c{i}", W1g_bf[i * 128:(i + 1) * 128, :])
    putb("w1cte", W1g_bf[512:520, :])
    putb("augw", colsum.reshape(1, HH))
    ind = np.zeros((128, 16 * NT), dtype=f32)
    w2i = np.zeros((128, 16 * NT), dtype=f32)
    for j in range(NT):
        ind[:, j * 16 + j] = 1.0
        w2i[:, j * 16 + j] = g["gh_w2"].reshape(-1)
    putb("indones", ind)
    putb("w2ind", w2i)
    putb("onesr", np.ones((1, 128), dtype=f32))
    iA = np.zeros((128, 64), dtype=f32)
    iB = np.zeros((128, 64), dtype=f32)
    for r in range(8):
        iA[:, r * 8 + r] = 1.0
        iB[:, r * 8 + r] = 1.0
    putb("indA", iA)
    putb("indB", iB)

    return dict(blobf=blobf, blobw=blobw, blobb=blobb), te, s1te, s2te, xm


def make_in_maps(inputs):
    wmap, te, s1te, s2te, xm = host_prep(inputs)
    f32 = np.float32
    rnn_f32 = np.concatenate([te, xm], axis=-1).astype(f32)   # (B, L, 10)
    rnn_hi = rnn_f32.astype(BF).astype(f32)
    rnn_lo = (rnn_f32 - rnn_hi).astype(BF)
    ones = np.ones((B, L, 1), np.float32)
    # rows: [hi(10); lo(10); 1; 1] sent over the wire; the device duplicates
    # hi into rows 22:32 to pair with blobw = [Whi; Wlo; c_hi; c_lo; Whi].
    rnn_all = np.concatenate([rnn_hi.astype(BF), rnn_lo,
                              ones.astype(BF), ones.astype(BF)],
                             axis=-1)                          # (B, L, 22)
    in_maps = []
    for i in range(N_CORES):
        m = dict(wmap)
        rnn_c = np.empty((SPC, 22, L), BF)
        st_c = np.empty((SPC, 8, 4 * T), BF)
        for s in range(SPC):
            bidx = i * SPC + s
            rnn_c[s] = rnn_all[bidx].T
            s1r = (-s1te[bidx] / OUT).reshape(NT, T)
            s2r = (s2te[bidx] / OUT).reshape(NT, T)
            st_c[s, :, 0:T] = s1r[0:8]
            st_c[s, :, T:2 * T] = s1r[8:16]
            st_c[s, :, 2 * T:3 * T] = s2r[0:8]
            st_c[s, :, 3 * T:4 * T] = s2r[8:16]
        m["rnn"] = np.ascontiguousarray(rnn_c)
        m["st"] = np.ascontiguousarray(st_c)
        in_maps.append(m)
    return in_maps


def _kernel_host(inputs):
    """Validated host fallback: same linear-recurrence formulation (numpy)."""
    f32 = np.float32
    g = {k: np.asarray(v, dtype=f32) for k, v in inputs.items()}

    def sig(z):
        out = np.exp(-np.abs(z))
        return np.where(z >= 0, 1.0 / (1.0 + out), out / (1.0 + out))

    xm = g["x"] * g["mask"][..., None]
    tshv = (g["t"] - g["t"][:, :1]) / g["time_scale"]
    h1 = np.maximum(tshv[..., None] * g["te_w1"][0] + g["te_b1"], 0.0)
    t_enc = (h1 @ g["te_w2"] + g["te_b2"]).astype(f32)
    rnn = np.concatenate([xm, t_enc], axis=-1)

    def scan(pw, pb, wz, bz, wh, bh, reverse):
        k = (rnn @ (pw @ wz) + (pb @ wz + bz)).astype(f32)
        v = (rnn @ (pw @ wh) + (pb @ wh + bh)).astype(f32)
        a = sig(-k)
        bv = sig(k) * np.where(v >= 0, v + 0.5, f32(np.exp(5.0)) * sig(v))
        if reverse:
            a = a[:, ::-1]; bv = bv[:, ::-1]
        h = np.empty_like(a)
        st = np.full((B, H), 0.5, dtype=f32)
        for i in range(L):
            st = a[:, i] * st + bv[:, i]
            h[:, i] = st
        return h[:, ::-1] if reverse else h

    hf = scan(g["fproj_w"], g["fproj_b"], g["f_wz"], g["f_bz"], g["f_wh"],
              g["f_bh"], False)
    hb = scan(g["bproj_w"], g["bproj_b"], g["b_wz"], g["b_bz"], g["b_wh"],
              g["b_bh"], True)
    X = np.concatenate([hf, hb, t_enc], axis=-1)
    mu = X.mean(-1, keepdims=True)
    var = ((X - mu) ** 2).mean(-1, keepdims=True)
    Xn = (X - mu) / np.sqrt(var + 1e-5) * g["ln_g"] + g["ln_b"]
    z = Xn @ g["gh_w1"] + g["gh_b1"]
    gel = 0.5 * z * (1.0 + np.tanh(f32(np.sqrt(2 / np.pi))
                                   * (z + f32(0.044715) * z ** 3)))
    return (gel @ g["gh_w2"] + g["gh_b2"]).astype(f32)




_RUNNER = None


def _get_runner():
    """Cached shard_map executable for the per-core program (the stock
    run_bass_kernel_spmd path rebuilds and retraces a fresh jax.jit on every
    call)."""
    global _RUNNER
    if _RUNNER is not None:
        return _RUNNER
    import jax
    import numpy as _np
    from jax.sharding import Mesh, PartitionSpec
    from jax.experimental.shard_map import shard_map
    from concourse import bass2jax, mybir as _mb
    from concourse.bass2jax import (_bass_exec_p, install_neuronx_cc_hook,
                                    partition_id_tensor)

    nc = _get_nc()
    install_neuronx_cc_hook()
    partition_name = (nc.partition_id_tensor.name
                      if nc.partition_id_tensor else None)
    in_names, out_names, out_avals, zero_outs = [], [], [], []
    for alloc in nc.m.functions[0].allocations:
        if not isinstance(alloc, _mb.MemoryLocationSet):
            continue
        name = alloc.memorylocations[0].name
        if alloc.kind == "ExternalInput":
            if name != partition_name:
                in_names.append(name)
        elif alloc.kind == "ExternalOutput":
            shape = tuple(alloc.tensor_shape)
            dtype = _mb.dt.np(alloc.dtype)
            out_names.append(name)
            out_avals.append(jax.core.ShapedArray(shape, dtype))
            zero_outs.append(_np.zeros(shape, dtype))
    n_params = len(in_names)
    n_outs = len(out_avals)
    in_names_all = list(in_names) + list(out_names)
    if partition_name is not None:
        in_names_all.append(partition_name)

    def _body(*args):
        operands = list(args)
        if partition_name is not None:
            operands.append(partition_id_tensor())
        outs = _bass_exec_p.bind(
            *operands, out_avals=tuple(out_avals), in_names=tuple(in_names_all),
            out_names=tuple(out_names), lowering_input_output_aliases=(),
            sim_require_finite=True, sim_require_nnan=True, nc=nc)
        return tuple(outs)

    devices = jax.devices()[:N_CORES]
    mesh = Mesh(_np.asarray(devices), ("core",))
    repl = {"blobf", "blobw", "blobb"}
    in_specs = tuple(PartitionSpec() if nm in repl else PartitionSpec("core")
                     for nm in in_names) + (PartitionSpec("core"),) * n_outs
    out_specs = (PartitionSpec("core"),) * n_outs
    sharded = jax.jit(
        shard_map(_body, mesh=mesh, in_specs=in_specs, out_specs=out_specs,
                  check_rep=False),
        donate_argnums=tuple(range(n_params, n_params + n_outs)),
        keep_unused=True)
    _RUNNER = (sharded, in_names, out_names, out_avals, zero_outs)
    return _RUNNER


def _run_cached(in_maps):
    import numpy as _np
    sharded, in_names, out_names, out_avals, zero_outs = _get_runner()
    repl = {"blobf", "blobw", "blobb"}
    concat_in = [_np.asarray(in_maps[0][nm]) if nm in repl else
                 _np.concatenate([_np.asarray(in_maps[c][nm])
                                  for c in range(N_CORES)], axis=0)
                 for nm in in_names]
    concat_zeros = [_np.zeros((N_CORES * z.shape[0], *z.shape[1:]), z.dtype)
                    for z in zero_outs]
    out_arrs = sharded(*concat_in, *concat_zeros)
    return [{nm: _np.asarray(out_arrs[i]).reshape(N_CORES,
                                                  *out_avals[i].shape)[c]
             for i, nm in enumerate(out_names)} for c in range(N_CORES)]


def kernel(**inputs) -> np.ndarray:
    in_maps = make_in_maps(inputs)
    results = _run_cached(in_maps)
    y = np.concatenate([results[i]["y"].reshape(SPC, L)
                        for i in range(N_CORES)], axis=0)
    b2 = np.float32(np.asarray(inputs["gh_b2"]).reshape(-1)[0])
    return (y + b2).reshape(B, L, 1).astype(np.float32)


if __name__ == "__main__":
    nc = build_core_program()
    print("built program")
